# revision 1
# baseline (speedup 1.0000x reference)
"""GNN message-passing encoder (GAT-style) on 8 Trainium2 NeuronCores.

Self-contained: hardcodes the problem shapes (N=100000, E=1600000, HN=64, L=3).

Sharding: core c owns dst nodes [c*NL, (c+1)*NL). Host sorts each core's
edges by (src_chunk, dst), packs whole dst-segments into 128-edge tiles
(<=31 segments/tile + trash slot 31), 32 tiles/block; src chunks of 32768
keep dma_gather's int16 indices in range.

Per layer, on device:
- table build: hlT = fc_w[l]^T h; a_s = W_src hl; a_d = W_dst hl + biases;
  write row tables to HBM; AllGather node table [a_s|hl] (512B fp32 rows).
- per block: dma_gather 4096 src rows + 1024 a_d-window rows (bf16);
  PE expand matmul (M_T x window; row 31 of M_T = ea, row 31 of window = u_l,
  fusing the ea*u edge term into the same contraction);
  alpha = expand + a_s; ACT Lrelu + Exp (softmax without max subtraction --
  safe here since |alpha| stays O(10)); contrib = [ex*hl | ex] in bf16;
  PE aggregation matmul (M^T x contrib) -> per-segment partial sums;
  dma_scatter_add into per-src-chunk-group accumulators.
- finalize: h[n] = num/(den + 1e-16); feed next layer or write output.
"""
import os
import sys

for _p in ("/opt/trn_rl_repo",):
    if _p not in sys.path:
        sys.path.insert(0, _p)

import numpy as np
import ml_dtypes

import concourse.bass as bass
import concourse.mybir as mybir
import concourse.tile as tile
from concourse import bacc
from concourse import library_config as libcfg
from concourse.bass_utils import run_bass_kernel_spmd

F32 = mybir.dt.float32
BF16 = mybir.dt.bfloat16
I16 = mybir.dt.int16
AX = mybir.AluOpType
AF = mybir.ActivationFunctionType
BF16NP = ml_dtypes.bfloat16


class Cfg:
    def __init__(self, N=100000, E=1600000, C=8, HN=64, IN_N=3, L=3, B=10,
                 CHUNK=32768, BT=32, NEG=0.2, EPS=1e-5):
        self.N, self.E, self.C, self.HN, self.IN_N, self.L, self.B = \
            N, E, C, HN, IN_N, L, B
        self.CHUNK, self.BT = CHUNK, BT
        self.NEG, self.EPS = NEG, EPS
        self.NL = N // C
        self.G = (N + CHUNK - 1) // CHUNK
        self.TE = 128
        self.SS = 32
        self.EB = self.TE * BT
        self.TRASH = self.NL        # accum trash row
        self.UROW = self.NL         # a_d table u-row
        self.NLP = -(-(self.NL + 1) // 128) * 128   # padded accum rows


# ------------------------------------------------------------- host prep ----
def _pack_idx16(vals):
    v = np.asarray(vals, dtype=np.int16)
    assert v.size % 16 == 0
    return np.tile(v.reshape(-1, 16).T, (8, 1)).copy()


def prep_core(cfg, src, dst, ea, c):
    NL, G, CHUNK, TE, SS = cfg.NL, cfg.G, cfg.CHUNK, cfg.TE, cfg.SS
    m = (dst >= c * NL) & (dst < (c + 1) * NL)
    src_c, dstl_c, ea_c = src[m], dst[m] - c * NL, ea[m]
    g_c = src_c // CHUNK
    out = []
    for g in range(G):
        mg = g_c == g
        sg, dg, eg = src_c[mg], dstl_c[mg], ea_c[mg]
        order = np.argsort(dg, kind="stable")
        sg, dg, eg = sg[order], dg[order], eg[order]
        if dg.size:
            bnd = np.flatnonzero(np.diff(dg)) + 1
            starts = np.concatenate([[0], bnd])
            ends = np.concatenate([bnd, [dg.size]])
        else:
            starts = ends = np.array([], dtype=np.int64)
        tiles, cur_e, cur_s, ne = [], [], [], 0
        for s0, s1 in zip(starts, ends):
            cnt = int(s1 - s0)
            assert cnt <= TE, f"segment larger than a tile: {cnt}"
            if ne + cnt > TE or len(cur_s) >= SS - 2:
                tiles.append((cur_e, cur_s))
                cur_e, cur_s, ne = [], [], 0
            cur_e.append((int(s0), int(s1)))
            cur_s.append(int(dg[s0]))
            ne += cnt
        if cur_s:
            tiles.append((cur_e, cur_s))
        out.append((tiles, sg, eg))
    return out


def build_streams(cfg, groups, nbg):
    G, CHUNK, TE, SS, BT, EB = cfg.G, cfg.CHUNK, cfg.TE, cfg.SS, cfg.BT, cfg.EB
    NB = sum(nbg)
    gidx = np.zeros(NB * EB, np.int16)
    earow = np.zeros((NB, 4, EB), np.float32)
    segslot = np.full((128, NB * BT), SS - 1, BF16NP)
    rep = np.full((NB, 128, EB), 99, BF16NP)
    for tib in range(BT):
        rep[:, 32 * (tib % 4):32 * (tib % 4) + 32, tib * TE:(tib + 1) * TE] = SS - 1
    adidx = np.zeros(NB * BT * SS, np.int16)
    scidx = np.full(NB * BT * SS, cfg.TRASH, np.int16)

    b0 = 0
    for g in range(G):
        tiles, sg, eg = groups[g]
        for ti in range(nbg[g] * BT):
            blk = b0 + ti // BT
            tib = ti % BT
            tcol = blk * BT + tib
            base_e = blk * EB + tib * TE
            base_s = tcol * SS
            adidx[base_s + 0] = cfg.UROW
            if ti < len(tiles):
                cur_e, cur_s = tiles[ti]
                p = 0
                for si, (s0, s1) in enumerate(cur_e):
                    n = s1 - s0
                    sl = slice(base_e + p, base_e + p + n)
                    gidx[sl] = (sg[s0:s1] - g * CHUNK).astype(np.int16)
                    qq = tib % 4
                    earow[blk, qq, tib * TE + p: tib * TE + p + n] = eg[s0:s1]
                    segslot[p:p + n, tcol] = si + 1
                    rep[blk, 32 * qq:32 * qq + 32,
                        tib * TE + p: tib * TE + p + n] = si + 1
                    p += n
                for si, dl in enumerate(cur_s):
                    adidx[base_s + 1 + si] = dl
                    scidx[base_s + 1 + si] = dl
        b0 += nbg[g]

    return dict(gidx=_pack_idx16(gidx), adidx=_pack_idx16(adidx),
                scidx=_pack_idx16(scidx), segslot=segslot, earow=earow, rep=rep)


def host_prepare(cfg, inputs):
    ei = np.asarray(inputs["edge_index"])
    src, dst = ei[0].astype(np.int64), ei[1].astype(np.int64)
    ea = np.asarray(inputs["edge_attr"], np.float32)[:, 0]
    per_core = [prep_core(cfg, src, dst, ea, c) for c in range(cfg.C)]
    nbg = [max(1, -(-max(len(pc[g][0]) for pc in per_core) // cfg.BT))
           for g in range(cfg.G)]

    HN, L = cfg.HN, cfg.L
    fnw = np.asarray(inputs["fc_node_w"], np.float32)
    fnb = np.asarray(inputs["fc_node_b"], np.float32)
    few = np.asarray(inputs["fc_edge_w"], np.float32)
    feb = np.asarray(inputs["fc_edge_b"], np.float32)
    gam = np.asarray(inputs["bn_gamma"], np.float32)
    bet = np.asarray(inputs["bn_beta"], np.float32)
    fcw = np.asarray(inputs["fc_w"], np.float32)
    fcb = np.asarray(inputs["fc_b"], np.float32)
    aw = np.asarray(inputs["attn_w"], np.float32)
    ab = np.asarray(inputs["attn_b"], np.float32)
    wdst, wsrc, we = aw[:, :HN, :], aw[:, HN:2 * HN, :], aw[:, 2 * HN:, :]
    u = np.stack([few[0] @ we[l] for l in range(L)])
    adb = np.stack([ab[l] + feb @ we[l] for l in range(L)])
    urow = np.zeros((L, 128), np.float32)
    urow[:, :HN] = u
    x = np.asarray(inputs["x"], np.float32)

    shared = dict(
        fnw=fnw, fnb=fnb.reshape(-1, 1), gam=gam.reshape(-1, 1),
        bet=bet.reshape(-1, 1),
        fcw=np.ascontiguousarray(fcw.transpose(1, 0, 2)),     # [64, L, 64]
        wsrc=np.ascontiguousarray(wsrc.transpose(1, 0, 2)),
        wdst=np.ascontiguousarray(wdst.transpose(1, 0, 2)),
        fcb=np.ascontiguousarray(fcb.T), adb=np.ascontiguousarray(adb.T),
        urow=urow,
        iota_mod=(np.arange(128) % 32).astype(BF16NP).reshape(128, 1),
        iota_bt=np.tile(np.arange(32, dtype=BF16NP), (128, cfg.BT)),
        ident=np.eye(128, dtype=np.float32),
    )
    in_maps = []
    for c in range(cfg.C):
        st = build_streams(cfg, per_core[c], nbg)
        im = dict(shared)
        im.update(st)
        im["xT"] = np.ascontiguousarray(x[c * cfg.NL:(c + 1) * cfg.NL].T)
        in_maps.append(im)
    return in_maps, nbg


# --------------------------------------------------------------- builder ----
def split_sync_waits(nc, max_waits=1):
    for f in nc.m.functions:
        for bb in f.blocks:
            old = bb.instructions
            if not any(i.sync_info and i.sync_info.on_wait
                       and len(i.sync_info.on_wait) > max_waits for i in old):
                continue
            new = []
            for ins in old:
                si = ins.sync_info
                if si is not None and si.on_wait and len(si.on_wait) > max_waits:
                    waits = list(si.on_wait)
                    extra, keep = waits[:-max_waits], waits[-max_waits:]
                    for j, w in enumerate(extra):
                        nop = mybir.InstNoOp(name=f"{ins.name}-wc{j}", ins=[], outs=[])
                        nop.engine = ins.engine
                        nop.sync_info = mybir.SyncInfo(on_wait=[w], on_update=[])
                        new.append(nop)
                    si.on_wait = keep
                new.append(ins)
            bb.instructions = new
    return nc


def build_nc(cfg, nbg):
    NL, G, CHUNK, TE, SS, BT, EB = (cfg.NL, cfg.G, cfg.CHUNK, cfg.TE, cfg.SS,
                                    cfg.BT, cfg.EB)
    HN, L, C, N, NLP = cfg.HN, cfg.L, cfg.C, cfg.N, cfg.NLP
    NB = sum(nbg)
    SLB = BT * SS
    SUB = min(8, BT)
    RG = [list(range(C))]
    gob = []
    for g in range(G):
        gob += [g] * nbg[g]

    nc = bacc.Bacc(None, target_bir_lowering=False, num_swdge_queues=4)
    din = {}

    def ext(name, shape, dt=F32):
        din[name] = nc.dram_tensor(name, shape, dt, kind="ExternalInput")

    ext("xT", [cfg.IN_N, NL])
    ext("gidx", [128, NB * EB // 16], I16)
    ext("adidx", [128, NB * SLB // 16], I16)
    ext("scidx", [128, NB * SLB // 16], I16)
    ext("segslot", [128, NB * BT], BF16)
    ext("earow", [NB, 4, EB])
    ext("rep", [NB, 128, EB], BF16)
    ext("fnw", [cfg.IN_N, HN]); ext("fnb", [HN, 1])
    ext("gam", [HN, 1]); ext("bet", [HN, 1])
    ext("fcw", [HN, L, HN]); ext("fcb", [HN, L])
    ext("wsrc", [HN, L, HN]); ext("wdst", [HN, L, HN]); ext("adb", [HN, L])
    ext("urow", [L, 128])
    ext("iota_mod", [128, 1], BF16); ext("iota_bt", [128, BT * SS], BF16)
    ext("ident", [128, 128])

    out = nc.dram_tensor("out", [NL, HN], F32, kind="ExternalOutput")
    local_table = nc.dram_tensor("local_table", [NL, 2 * HN], F32)
    full_table = nc.dram_tensor("full_table", [N, 2 * HN], F32,
                                addr_space="Shared")
    local_ad = nc.dram_tensor("local_ad", [NL + 1, 128], F32)
    accum = nc.dram_tensor("accum", [2 * G, NLP, 2 * HN], F32)
    bn_loc = nc.dram_tensor("bn_loc", [HN, 2], F32)
    bn_sh = nc.dram_tensor("bn_sh", [HN, 2], F32, addr_space="Shared")

    with tile.TileContext(nc) as tc:
        with tc.tile_pool(name="persist", bufs=1) as pp:
            nc.gpsimd.load_library(libcfg.mlp)
            hT = pp.tile([HN, NL], F32)
            w = {}
            for nm, shp, dt in (
                ("fnw", [cfg.IN_N, HN], F32), ("fnb", [HN, 1], F32),
                ("gam", [HN, 1], F32), ("bet", [HN, 1], F32),
                ("fcw", [HN, L, HN], F32), ("fcb", [HN, L], F32),
                ("wsrc", [HN, L, HN], F32), ("wdst", [HN, L, HN], F32),
                ("adb", [HN, L], F32),
                ("iota_mod", [128, 1], BF16), ("iota_bt", [128, BT * SS], BF16),
                ("ident", [128, 128], F32),
                ("segslot", [128, NB * BT], BF16),
            ):
                w[nm] = pp.tile(shp, dt, tag=nm, name=nm)
                nc.sync.dma_start(out=w[nm][:], in_=din[nm][:])
            zero2k = pp.tile([128, 2048], F32)
            nc.vector.memset(zero2k[:], 0.0)
            ident = w["ident"]

            # ---------------- input + BN ----------------
            with tc.tile_pool(name="bn", bufs=2) as bp, \
                 tc.tile_pool(name="bnsq", bufs=1) as bq, \
                 tc.tile_pool(name="bnp", bufs=2, space="PSUM") as bpp:
                xt = bq.tile([cfg.IN_N, NL], F32, tag="xt")
                nc.sync.dma_start(out=xt[:], in_=din["xT"][:])
                for ci in range(-(-NL // 512)):
                    c0 = ci * 512
                    n = min(512, NL - c0)
                    ps = bpp.tile([HN, 512], F32, tag="ps")
                    nc.tensor.matmul(out=ps[:, :n], lhsT=w["fnw"][:],
                                     rhs=xt[:, c0:c0 + n], start=True, stop=True)
                    nc.scalar.copy(out=hT[:, c0:c0 + n], in_=ps[:, :n])
                nc.vector.tensor_scalar_add(hT[:], hT[:], w["fnb"][:])
                st = bp.tile([HN, 2], F32, tag="st")
                sq = bq.tile([HN, NL], F32, tag="sq")
                nc.vector.reduce_sum(st[:, 0:1], hT[:], axis=mybir.AxisListType.X)
                nc.vector.scalar_tensor_tensor(out=sq[:], in0=hT[:], scalar=1.0,
                                               in1=hT[:], op0=AX.mult, op1=AX.mult,
                                               accum_out=st[:, 1:2])
                nc.sync.dma_start(out=bn_loc[:], in_=st[:])
                nc.gpsimd.collective_compute("AllReduce", AX.add,
                                             replica_groups=RG,
                                             ins=[bn_loc[:]], outs=[bn_sh[:]])
                sg = bp.tile([HN, 2], F32, tag="sg")
                nc.sync.dma_start(out=sg[:], in_=bn_sh[:])
                mean = bp.tile([HN, 1], F32, tag="mean")
                var = bp.tile([HN, 1], F32, tag="var")
                nc.vector.tensor_scalar_mul(mean[:], sg[:, 0:1], 1.0 / N)
                nc.vector.tensor_scalar_mul(var[:], sg[:, 1:2], 1.0 / N)
                msq = bp.tile([HN, 1], F32, tag="msq")
                nc.vector.tensor_mul(msq[:], mean[:], mean[:])
                nc.vector.tensor_sub(var[:], var[:], msq[:])
                nc.vector.tensor_scalar_add(var[:], var[:], cfg.EPS)
                rs = bp.tile([HN, 1], F32, tag="rs")
                nc.scalar.activation(out=rs[:], in_=var[:], func=AF.Sqrt)
                nc.vector.reciprocal(rs[:], rs[:])
                scale = bp.tile([HN, 1], F32, tag="scale")
                nc.vector.tensor_mul(scale[:], rs[:], w["gam"][:])
                nbias = bp.tile([HN, 1], F32, tag="nbias")
                nc.vector.tensor_mul(nbias[:], mean[:], scale[:])
                nc.vector.scalar_tensor_tensor(out=nbias[:], in0=nbias[:],
                                               scalar=-1.0, in1=w["bet"][:],
                                               op0=AX.mult, op1=AX.add)
                nc.vector.tensor_scalar(out=hT[:], in0=hT[:], scalar1=scale[:],
                                        scalar2=nbias[:], op0=AX.mult, op1=AX.add)

            # ---------------- layers ----------------
            for l in range(L):
                with tc.tile_pool(name=f"tb{l}", bufs=2) as tp, \
                     tc.tile_pool(name=f"tp{l}", bufs=2, space="PSUM") as tpp:
                    for ci in range(-(-NL // 512)):
                        c0 = ci * 512
                        n = min(512, NL - c0)
                        psh = tpp.tile([HN, 512], F32, tag="psh")
                        nc.tensor.matmul(out=psh[:, :n], lhsT=w["fcw"][:, l, :],
                                         rhs=hT[:, c0:c0 + n], start=True, stop=True)
                        hl = tp.tile([HN, 512], F32, tag="hl")
                        nc.scalar.copy(out=hl[:, :n], in_=psh[:, :n])
                        nc.vector.tensor_scalar_add(hl[:, :n], hl[:, :n],
                                                    w["fcb"][:, l:l + 1])
                        pss = tpp.tile([HN, 512], F32, tag="psh")
                        nc.tensor.matmul(out=pss[:, :n], lhsT=w["wsrc"][:, l, :],
                                         rhs=hl[:, :n], start=True, stop=True)
                        asb = tp.tile([HN, 512], F32, tag="asb")
                        nc.scalar.copy(out=asb[:, :n], in_=pss[:, :n])
                        psd = tpp.tile([HN, 512], F32, tag="psh")
                        nc.tensor.matmul(out=psd[:, :n], lhsT=w["wdst"][:, l, :],
                                         rhs=hl[:, :n], start=True, stop=True)
                        adt = tp.tile([HN, 512], F32, tag="adt")
                        nc.vector.tensor_scalar_add(adt[:, :n], psd[:, :n],
                                                    w["adb"][:, l:l + 1])
                        for si in range(-(-n // 128)):
                            s0, sn = si * 128, min(128, n - si * 128)
                            rows = tp.tile([128, 2 * HN], F32, tag="rows")
                            pt = tpp.tile([128, HN], F32, tag="pt")
                            nc.tensor.transpose(out=pt[:sn, :],
                                                in_=asb[:, s0:s0 + sn],
                                                identity=ident[:HN, :HN])
                            nc.scalar.copy(out=rows[:sn, 0:HN], in_=pt[:sn, :])
                            pt2 = tpp.tile([128, HN], F32, tag="pt")
                            nc.tensor.transpose(out=pt2[:sn, :],
                                                in_=hl[:, s0:s0 + sn],
                                                identity=ident[:HN, :HN])
                            nc.scalar.copy(out=rows[:sn, HN:2 * HN], in_=pt2[:sn, :])
                            nc.sync.dma_start(
                                out=local_table[c0 + s0:c0 + s0 + sn, :],
                                in_=rows[:sn, :])
                            adr = tp.tile([128, 128], F32, tag="adr")
                            nc.vector.memset(adr[:], 0.0)
                            pt3 = tpp.tile([128, HN], F32, tag="pt")
                            nc.tensor.transpose(out=pt3[:sn, :],
                                                in_=adt[:, s0:s0 + sn],
                                                identity=ident[:HN, :HN])
                            nc.vector.tensor_copy(adr[:sn, 0:HN], pt3[:sn, :])
                            nc.sync.dma_start(
                                out=local_ad[c0 + s0:c0 + s0 + sn, :],
                                in_=adr[:sn, :])
                    nc.sync.dma_start(out=local_ad[NL:NL + 1, :],
                                      in_=din["urow"][l:l + 1, :])
                    # zero accums
                    for g in range(2 * G):
                        a_tot = NLP // 128
                        off = 0
                        while off < a_tot:
                            an = min(14, a_tot - off)
                            nc.sync.dma_start(
                                out=accum[g].rearrange("(a p) f -> p a f", p=128)
                                [:, off:off + an, :],
                                in_=zero2k[:, :an * 128].rearrange(
                                    "p (a f) -> p a f", f=2 * HN))
                            off += an

                nc.gpsimd.collective_compute("AllGather", AX.bypass,
                                             replica_groups=RG,
                                             ins=[local_table[:]],
                                             outs=[full_table[:]])

                with tc.tile_pool(name=f"ep{l}", bufs=2) as ep, \
                     tc.tile_pool(name=f"pp{l}", bufs=2, space="PSUM") as epp:
                    for b in range(NB):
                        g = gob[b]
                        gix = ep.tile([128, EB // 16], I16, tag="gix")
                        nc.sync.dma_start(
                            out=gix[:],
                            in_=din["gidx"][:, b * (EB // 16):(b + 1) * (EB // 16)])
                        aix = ep.tile([128, SLB // 16], I16, tag="aix")
                        nc.sync.dma_start(
                            out=aix[:],
                            in_=din["adidx"][:, b * (SLB // 16):(b + 1) * (SLB // 16)])
                        six = ep.tile([128, SLB // 16], I16, tag="six")
                        nc.sync.dma_start(
                            out=six[:],
                            in_=din["scidx"][:, b * (SLB // 16):(b + 1) * (SLB // 16)])
                        srcr = ep.tile([128, BT, 2 * HN], F32, tag="srcr")
                        for hh in range(2):
                            nc.gpsimd.dma_gather(
                                out_ap=srcr[:, hh * (BT // 2):(hh + 1) * (BT // 2), :],
                                in_ap=full_table[g * CHUNK:min((g + 1) * CHUNK, N), :],
                                idxs_ap=gix[:, hh * (EB // 32):(hh + 1) * (EB // 32)],
                                num_idxs=EB // 2, num_idxs_reg=EB // 2,
                                elem_size=2 * HN,
                                single_packet=False, queue_num=(b + 2 * hh) % 4)
                        adw = ep.tile([128, SLB // 128, 128], F32, tag="adw")
                        nc.gpsimd.dma_gather(
                            out_ap=adw[:], in_ap=local_ad[:],
                            idxs_ap=aix[:],
                            num_idxs=SLB, num_idxs_reg=SLB, elem_size=128,
                            single_packet=False, queue_num=(b + 1) % 4)
                        rp = ep.tile([128, EB], BF16, tag="rp")
                        nc.sync.dma_start(out=rp[:], in_=din["rep"][b, :, :])
                        mt = ep.tile([128, EB], F32, tag="mt", bufs=1)
                        nc.vector.tensor_tensor(
                            out=mt[:], in0=w["iota_mod"][:].to_broadcast([128, EB]),
                            in1=rp[:], op=AX.is_equal)
                        for qq in range(4):
                            nc.sync.dma_start(
                                out=mt[32 * qq:32 * qq + 1, :],
                                in_=din["earow"][b, qq:qq + 1, :])

                        mm = ep.tile([128, BT, SS], F32, tag="mm", bufs=1)
                        ssl = w["segslot"][:, b * BT:(b + 1) * BT]
                        nc.vector.tensor_tensor(
                            out=mm[:],
                            in0=ssl.to_broadcast([128, BT, SS]),
                            in1=w["iota_bt"][:].rearrange(
                                "p (t s) -> p t s", s=SS),
                            op=AX.is_equal)
                        alpha = ep.tile([128, BT, HN], F32, tag="alpha")
                        contrib = ep.tile([128, BT, 2 * HN], F32, tag="contrib")
                        scat = ep.tile([128, SLB // 128, 2 * HN], F32, tag="scat")
                        for ww in range(BT // SUB):
                            psx = epp.tile([128, SUB * HN], F32, tag="psx")
                            for t2 in range(SUB):
                                t = ww * SUB + t2
                                po = (SS * t) % 128
                                nc.tensor.matmul(
                                    out=psx[:, t2 * HN:(t2 + 1) * HN],
                                    lhsT=mt[:, t * TE:(t + 1) * TE],
                                    rhs=adw[:, t // 4, 0:HN],
                                    start=True, stop=True)
                            nc.vector.tensor_tensor(
                                out=alpha[:, ww * SUB:(ww + 1) * SUB, :],
                                in0=psx[:].rearrange("p (t h) -> p t h", h=HN),
                                in1=srcr[:, ww * SUB:(ww + 1) * SUB, 0:HN],
                                op=AX.add)
                        nc.vector.scalar_tensor_tensor(
                            out=alpha[:], in0=alpha[:], scalar=cfg.NEG,
                            in1=alpha[:], op0=AX.mult, op1=AX.max)
                        nc.scalar.activation(out=contrib[:, :, HN:2 * HN],
                                             in_=alpha[:], func=AF.Exp)
                        nc.vector.tensor_mul(contrib[:, :, 0:HN],
                                             contrib[:, :, HN:2 * HN],
                                             srcr[:, :, HN:2 * HN])
                        for q in range(BT // 2):
                            psa = epp.tile([64, 2 * HN], F32, tag="psa")
                            for t2 in range(2):
                                t = q * 2 + t2
                                nc.tensor.matmul(
                                    out=psa[t2 * SS:(t2 + 1) * SS, :],
                                    lhsT=mm[:, t, :], rhs=contrib[:, t, :],
                                    start=True, stop=True)
                            nc.scalar.copy(
                                out=scat[64 * (q % 2):64 * (q % 2) + 64,
                                         q // 2, :],
                                in_=psa[:])
                        nc.gpsimd.dma_scatter_add(
                            accum[2 * g + (b % 2), :NL + 1, :], scat[:], six[:],
                            SLB, SLB, 2 * HN,
                            single_packet=False, queue_num=(b + 2) % 4)

                with tc.tile_pool(name=f"fi{l}", bufs=3) as fp, \
                     tc.tile_pool(name=f"fp{l}", bufs=3, space="PSUM") as fpp:
                    for ci in range(-(-NL // 128)):
                        r0 = ci * 128
                        n = min(128, NL - r0)
                        a4 = fp.tile([128, 2 * G, 2 * HN], F32, tag="a4")
                        nc.sync.dma_start(
                            out=a4[:n, :, :],
                            in_=accum[:, r0:r0 + n, :].rearrange("g n d -> n g d"))
                        s01 = fp.tile([128, 2 * HN], F32, tag="s01")
                        nc.vector.tensor_add(s01[:n, :], a4[:n, 0, :], a4[:n, 1, :])
                        for gg in range(2, 2 * G):
                            nc.vector.tensor_add(s01[:n, :], s01[:n, :],
                                                 a4[:n, gg, :])
                        pt = fpp.tile([128, 128], F32, tag="pt")
                        nc.tensor.transpose(out=pt[:, :n], in_=s01[:n, :],
                                            identity=ident[:n, :n])
                        den = fp.tile([HN, 128], F32, tag="den")
                        nc.vector.tensor_scalar_add(den[:, :n],
                                                    pt[HN:2 * HN, :n], 1e-16)
                        nc.vector.reciprocal(den[:, :n], den[:, :n])
                        if l < L - 1:
                            nc.vector.tensor_mul(hT[:, r0:r0 + n], pt[0:HN, :n],
                                                 den[:, :n])
                        else:
                            hout = fp.tile([HN, 128], F32, tag="hout")
                            nc.vector.tensor_mul(hout[:, :n], pt[0:HN, :n],
                                                 den[:, :n])
                            po = fpp.tile([128, HN], F32, tag="po")
                            nc.tensor.transpose(out=po[:n, :], in_=hout[:, :n],
                                                identity=ident[:HN, :HN])
                            orow = fp.tile([128, HN], F32, tag="orow")
                            nc.scalar.copy(out=orow[:n, :], in_=po[:n, :])
                            nc.sync.dma_start(out=out[r0:r0 + n, :],
                                              in_=orow[:n, :])
    return nc


# --------------------------------------------------------------- entry ------
def kernel(**inputs):
    cfg = Cfg()
    in_maps, nbg = host_prepare(cfg, inputs)
    nc = build_nc(cfg, nbg)
    nc.compile()
    split_sync_waits(nc)
    res = run_bass_kernel_spmd(nc, in_maps, core_ids=list(range(cfg.C)),
                               trace=bool(int(os.environ.get("GNN_TRACE", "0"))))
    if res.exec_time_ns is not None:
        print(f"HW exec time: {res.exec_time_ns} ns")
    h = np.concatenate([np.asarray(res.results[c]["out"])
                        for c in range(cfg.C)], axis=0)
    return h.reshape(cfg.B, -1, cfg.HN)



# revision 2
# speedup vs baseline: 1.0876x; 1.0876x over previous
"""GNN message-passing encoder (GAT-style) on 8 Trainium2 NeuronCores — v4.

Self-contained: hardcodes the problem shapes (N=100000, E=1600000, HN=64, L=3).

Design (edge-major, bf16, scatterless):
- Core c owns dst nodes [c*NL, (c+1)*NL). Node space re-keyed into G=4 chunks:
  chunk s = slice s of every core (SP=3200 padded rows per core-slice), so each
  chunk's table is completed by ONE sub-AllGather -> gather of chunk g overlaps
  the AllGather of chunk g+1.
- Per layer: table build emits node-major bf16 rows [a_s | hl] per slice,
  4 sub-AllGathers; a_d table kept local node-major (127-node subchunks,
  row 127 = u_l so the ea*u edge term rides the same expand matmul).
- Edges sorted by (chunk, dst-subchunk, dst); tiles of 128 edges never span a
  (chunk, subchunk); blocks of 32 tiles never span a chunk. Per block:
  dma_gather 4096 src rows (256B bf16); one-hot masks od (dst-major, via K=1
  broadcast matmul + is_eq) and odT (edge-major, via is_eq against iota);
  expand matmul a_d+ea*u per tile; alpha = psx + a_s; LeakyReLU; Exp;
  contrib = [ex*hl | ex]; aggregation matmul accumulates [128 dst, 128 ch]
  PSUM across the subchunk's tiles; flushed by DVE add into an SBUF
  accumulator — no scatter, no HBM accum round-trip.
- Finalize: h = num/(den+1e-16) from SBUF, PE-transpose back to channel-major
  for the next layer; final layer DMAs node-major rows out.
"""
import os
import sys

for _p in ("/opt/trn_rl_repo",):
    if _p not in sys.path:
        sys.path.insert(0, _p)

import numpy as np
import ml_dtypes

import concourse.bass as bass
import concourse.mybir as mybir
import concourse.tile as tile
from concourse import bacc
from concourse import library_config as libcfg
from concourse.bass_utils import run_bass_kernel_spmd

F32 = mybir.dt.float32
BF16 = mybir.dt.bfloat16
I16 = mybir.dt.int16
AX = mybir.AluOpType
AF = mybir.ActivationFunctionType
BF16NP = ml_dtypes.bfloat16
FP8 = mybir.dt.float8e4
ODT_NP = ml_dtypes.float8_e4m3


class Cfg:
    def __init__(self, N=100000, E=1600000, BT=32, B=10):
        self.N, self.E, self.C, self.HN, self.IN_N, self.L, self.B = \
            N, E, 8, 64, 3, 3, B
        self.NEG, self.EPS = 0.2, 1e-5
        self.NL = self.N // self.C          # 12500 local dst nodes
        self.G = 4                          # src chunks
        self.S = self.NL // self.G          # 3125 real rows per slice
        self.SP = -(-self.S // 128) * 128   # 3200 padded slice rows
        self.CH = self.C * self.SP          # 25600 chunk rows
        self.RP = self.SP // 128            # staging row-groups per slice
        self.SC = 127                       # dst nodes per subchunk
        self.NSC = -(-self.NL // self.SC)   # 99 subchunks
        self.TE = 128
        self.BT = BT
        self.SUB = min(8, BT)
        self.EB = self.TE * BT              # 4096 edges per block
        assert self.NL == self.G * self.S and self.CH <= 32768


# ------------------------------------------------------------- host prep ----
def _pack_idx16(vals):
    v = np.asarray(vals, dtype=np.int16)
    assert v.size % 16 == 0
    return np.tile(v.reshape(-1, 16).T, (8, 1)).copy()


def prep_edges(cfg, src, dst, ea):
    """Per-core edge partition grouped by (g, sc); returns per-core dict of
    (g, sc) -> (srcidx array, code array, ea array)."""
    per_core = []
    for c in range(cfg.C):
        m = (dst >= c * cfg.NL) & (dst < (c + 1) * cfg.NL)
        s_c, d_c, e_c = src[m], dst[m] - c * cfg.NL, ea[m]
        g = (s_c % cfg.NL) // cfg.S
        sidx = (s_c // cfg.NL) * cfg.SP + (s_c % cfg.S)
        sc = d_c // cfg.SC
        code = d_c % cfg.SC
        order = np.lexsort((d_c, sc, g))
        per_core.append(dict(g=g[order], sidx=sidx[order], sc=sc[order],
                             code=code[order], ea=e_c[order]))
    return per_core


def build_tiles(cfg, per_core):
    """Global tile layout (shared across cores) + per-core packed streams."""
    C, G, NSC, TE, BT = cfg.C, cfg.G, cfg.NSC, cfg.TE, cfg.BT
    # edge counts per (core, g, sc)
    cnt = np.zeros((C, G, NSC), np.int64)
    for c in range(C):
        pc = per_core[c]
        np.add.at(cnt[c], (pc["g"], pc["sc"]), 1)
    T = np.maximum(cnt, 0).max(axis=0)          # [G, NSC] max count
    T = -(-T // TE)                              # tiles per (g, sc)
    # pad each g to a multiple of BT (extend last subchunk's tile count)
    for g in range(G):
        rem = (-T[g].sum()) % BT
        T[g, NSC - 1] += rem
    NBg = [int(T[g].sum()) // BT for g in range(G)]
    TT = int(T.sum())
    NB = sum(NBg)
    assert TT == NB * BT

    # shared per-tile metadata
    tile_g = np.zeros(TT, np.int32)
    tile_sc = np.zeros(TT, np.int32)
    tile_st = np.zeros(TT, bool)
    tile_sp = np.zeros(TT, bool)
    t = 0
    for g in range(G):
        for sc in range(NSC):
            n = int(T[g, sc])
            if n == 0:
                continue
            tile_g[t:t + n] = g
            tile_sc[t:t + n] = sc
            tile_st[t] = True
            tile_sp[t + n - 1] = True
            t += n
    assert t == TT

    # per-core streams
    streams = []
    d127 = np.arange(127)
    for c in range(C):
        pc = per_core[c]
        gidx = np.zeros(TT * TE, np.int16)
        codeF = np.full(TT * TE, -1, np.int32)
        earow = np.zeros(TT * TE, np.float32)
        # group start offsets in the sorted stream
        t0 = 0
        p = 0  # pointer into sorted edge stream
        for g in range(G):
            for sc in range(NSC):
                n_edges = int(cnt[c, g, sc])
                ntiles = int(T[g, sc])
                base = t0 * TE
                sl = slice(p, p + n_edges)
                gidx[base:base + n_edges] = pc["sidx"][sl].astype(np.int16)
                codeF[base:base + n_edges] = pc["code"][sl]
                earow[base:base + n_edges] = pc["ea"][sl]
                p += n_edges
                t0 += ntiles
        assert p == len(pc["g"])
        # od[d, e] one-hot (dst-major) with ea on row 127; odT[p, t, d]
        od = np.zeros((128, TT * TE), BF16NP)
        od[:127, :] = (codeF[None, :] == d127[:, None]).astype(BF16NP)
        od[127, :] = earow.astype(BF16NP)
        odT = (codeF.reshape(TT, TE).T[:, :, None]
               == d127[None, None, :]).astype(ODT_NP)   # [p, TT, 127]
        odTf = np.zeros((128, TT, 128), ODT_NP)
        odTf[:, :, :127] = odT
        streams.append(dict(
            gidx=_pack_idx16(gidx),
            odh=od.reshape(128, NB, cfg.EB).transpose(1, 0, 2).copy(),
            odth=odTf.reshape(128, NB, BT * 128).transpose(1, 0, 2).copy(),
        ))
    meta = dict(NB=NB, NBg=NBg, TT=TT, tile_g=tile_g, tile_sc=tile_sc,
                tile_st=tile_st, tile_sp=tile_sp)
    return meta, streams


def host_prepare(cfg, inputs):
    ei = np.asarray(inputs["edge_index"])
    src, dst = ei[0].astype(np.int64), ei[1].astype(np.int64)
    ea = np.asarray(inputs["edge_attr"], np.float32)[:, 0]
    per_core = prep_edges(cfg, src, dst, ea)
    meta, streams = build_tiles(cfg, per_core)

    HN, L = cfg.HN, cfg.L
    fnw = np.asarray(inputs["fc_node_w"], np.float32)
    fnb = np.asarray(inputs["fc_node_b"], np.float32)
    few = np.asarray(inputs["fc_edge_w"], np.float32)
    feb = np.asarray(inputs["fc_edge_b"], np.float32)
    gam = np.asarray(inputs["bn_gamma"], np.float32)
    bet = np.asarray(inputs["bn_beta"], np.float32)
    fcw = np.asarray(inputs["fc_w"], np.float32)
    fcb = np.asarray(inputs["fc_b"], np.float32)
    aw = np.asarray(inputs["attn_w"], np.float32)
    ab = np.asarray(inputs["attn_b"], np.float32)
    wdst, wsrc, we = aw[:, :HN, :], aw[:, HN:2 * HN, :], aw[:, 2 * HN:, :]
    u = np.stack([few[0] @ we[l] for l in range(L)])          # [L, 64]
    adb = np.stack([ab[l] + feb @ we[l] for l in range(L)])   # [L, 64]
    x = np.asarray(inputs["x"], np.float32)

    shared = dict(
        fnw=fnw, fnb=fnb.reshape(-1, 1), gam=gam.reshape(-1, 1),
        bet=bet.reshape(-1, 1),
        fcw=np.ascontiguousarray(fcw.transpose(1, 0, 2)).astype(BF16NP),
        wsrc=np.ascontiguousarray(wsrc.transpose(1, 0, 2)).astype(BF16NP),
        wdst=np.ascontiguousarray(wdst.transpose(1, 0, 2)).astype(BF16NP),
        fcb=np.ascontiguousarray(fcb.T), adb=np.ascontiguousarray(adb.T),
        urow=u.astype(BF16NP),                                 # [L, 64]
        urow_b=np.broadcast_to(u.astype(BF16NP)[:, None, :],
                               (L, cfg.NSC, HN)).copy(),
        identb=np.eye(128, dtype=BF16NP),
    )
    in_maps = []
    for c in range(cfg.C):
        im = dict(shared)
        im.update(streams[c])
        im["xT"] = np.ascontiguousarray(
            x[c * cfg.NL:(c + 1) * cfg.NL].T).astype(BF16NP)
        in_maps.append(im)
    return in_maps, meta


# --------------------------------------------------------------- builder ----
def split_sync_waits(nc, max_waits=1):
    for f in nc.m.functions:
        for bb in f.blocks:
            old = bb.instructions
            if not any(i.sync_info and i.sync_info.on_wait
                       and len(i.sync_info.on_wait) > max_waits for i in old):
                continue
            new = []
            for ins in old:
                si = ins.sync_info
                if si is not None and si.on_wait and len(si.on_wait) > max_waits:
                    waits = list(si.on_wait)
                    extra, keep = waits[:-max_waits], waits[-max_waits:]
                    for j, w in enumerate(extra):
                        nop = mybir.InstNoOp(name=f"{ins.name}-wc{j}", ins=[], outs=[])
                        nop.engine = ins.engine
                        nop.sync_info = mybir.SyncInfo(on_wait=[w], on_update=[])
                        new.append(nop)
                    si.on_wait = keep
                new.append(ins)
            bb.instructions = new
    return nc


def build_nc(cfg, meta):
    NL, G, S, SP, CH = cfg.NL, cfg.G, cfg.S, cfg.SP, cfg.CH
    SC, NSC, TE, BT, EB = cfg.SC, cfg.NSC, cfg.TE, cfg.BT, cfg.EB
    HN, L, C, N = cfg.HN, cfg.L, cfg.C, cfg.N
    NB, NBg, TT = meta["NB"], meta["NBg"], meta["TT"]
    tile_g, tile_sc = meta["tile_g"], meta["tile_sc"]
    tile_st, tile_sp = meta["tile_st"], meta["tile_sp"]
    RG = [list(range(C))]

    nc = bacc.Bacc(None, target_bir_lowering=False, num_swdge_queues=4)
    din = {}

    def ext(name, shape, dt=F32):
        din[name] = nc.dram_tensor(name, shape, dt, kind="ExternalInput")

    ext("xT", [cfg.IN_N, NL], BF16)
    ext("gidx", [128, TT * TE // 16], I16)
    ext("odh", [NB, 128, EB], BF16)
    ext("odth", [NB, 128, BT * 128], FP8)
    ext("fnw", [cfg.IN_N, HN]); ext("fnb", [HN, 1])
    ext("gam", [HN, 1]); ext("bet", [HN, 1])
    ext("fcw", [HN, L, HN], BF16); ext("fcb", [HN, L])
    ext("wsrc", [HN, L, HN], BF16); ext("wdst", [HN, L, HN], BF16)
    ext("adb", [HN, L])
    ext("urow", [L, HN], BF16)
    ext("urow_b", [L, NSC, HN], BF16)
    ext("identb", [128, 128], BF16)

    out = nc.dram_tensor("out", [NL, HN], F32, kind="ExternalOutput")
    lt = [nc.dram_tensor(f"lt{s}", [SP, 128], BF16) for s in range(G)]
    fsh = [nc.dram_tensor(f"fsh{s}", [C, SP, 128], BF16, addr_space="Shared")
           for s in range(G)]
    bn_loc = nc.dram_tensor("bn_loc", [HN, 2], F32)
    bn_sh = nc.dram_tensor("bn_sh", [HN, 2], F32, addr_space="Shared")

    with tile.TileContext(nc) as tc:
        with tc.tile_pool(name="persist", bufs=1) as pp:
            nc.gpsimd.load_library(libcfg.mlp)
            w = {}
            for nm, shp, dt in (
                ("fnw", [cfg.IN_N, HN], F32), ("fnb", [HN, 1], F32),
                ("gam", [HN, 1], F32), ("bet", [HN, 1], F32),
                ("fcw", [HN, L, HN], BF16), ("fcb", [HN, L], F32),
                ("wsrc", [HN, L, HN], BF16), ("wdst", [HN, L, HN], BF16),
                ("adb", [HN, L], F32), ("urow", [L, HN], BF16),
                ("identb", [128, 128], BF16),
            ):
                w[nm] = pp.tile(shp, dt, tag=nm, name=nm)
                nc.sync.dma_start(out=w[nm][:], in_=din[nm][:])
            ident = w["identb"]
            hTb = pp.tile([HN, NL], BF16, tag="hTb", name="hTb")
            ad_nm = pp.tile([128, NSC, HN], BF16, tag="ad_nm", name="ad_nm")
            accum = pp.tile([128, NSC, 128], BF16, tag="accum", name="accum")

            # ---------------- input proj + BN (f32) ----------------
            with tc.tile_pool(name="bn", bufs=2) as bp, \
                 tc.tile_pool(name="bnsq", bufs=1) as bq, \
                 tc.tile_pool(name="bnp", bufs=2, space="PSUM") as bpp:
                xt = bq.tile([cfg.IN_N, NL], BF16, tag="xt")
                nc.sync.dma_start(out=xt[:], in_=din["xT"][:])
                fnwb = bp.tile([cfg.IN_N, HN], BF16, tag="fnwb")
                nc.vector.tensor_copy(fnwb[:], w["fnw"][:])
                for ci in range(-(-NL // 512)):
                    c0 = ci * 512
                    n = min(512, NL - c0)
                    ps = bpp.tile([HN, 512], F32, tag="ps")
                    nc.tensor.matmul(out=ps[:, :n], lhsT=fnwb[:],
                                     rhs=xt[:, c0:c0 + n], start=True, stop=True)
                    nc.vector.tensor_scalar_add(hTb[:, c0:c0 + n], ps[:, :n],
                                                w["fnb"][:])
                st = bp.tile([HN, 2], F32, tag="st")
                stp = bp.tile([HN, cfg.G], F32, tag="stp")
                sqc = bq.tile([HN, S], F32, tag="sqc")
                nc.vector.reduce_sum(st[:, 0:1], hTb[:], axis=mybir.AxisListType.X)
                for j in range(cfg.G):
                    nc.vector.scalar_tensor_tensor(
                        out=sqc[:], in0=hTb[:, j * S:(j + 1) * S], scalar=1.0,
                        in1=hTb[:, j * S:(j + 1) * S], op0=AX.mult, op1=AX.mult,
                        accum_out=stp[:, j:j + 1])
                nc.vector.reduce_sum(st[:, 1:2], stp[:], axis=mybir.AxisListType.X)
                nc.sync.dma_start(out=bn_loc[:], in_=st[:])
                nc.gpsimd.collective_compute("AllReduce", AX.add,
                                             replica_groups=RG,
                                             ins=[bn_loc[:]], outs=[bn_sh[:]])
                sg = bp.tile([HN, 2], F32, tag="sg")
                nc.sync.dma_start(out=sg[:], in_=bn_sh[:])
                mean = bp.tile([HN, 1], F32, tag="mean")
                var = bp.tile([HN, 1], F32, tag="var")
                nc.vector.tensor_scalar_mul(mean[:], sg[:, 0:1], 1.0 / N)
                nc.vector.tensor_scalar_mul(var[:], sg[:, 1:2], 1.0 / N)
                msq = bp.tile([HN, 1], F32, tag="msq")
                nc.vector.tensor_mul(msq[:], mean[:], mean[:])
                nc.vector.tensor_sub(var[:], var[:], msq[:])
                nc.vector.tensor_scalar_add(var[:], var[:], cfg.EPS)
                rs = bp.tile([HN, 1], F32, tag="rs")
                nc.scalar.activation(out=rs[:], in_=var[:], func=AF.Sqrt)
                nc.vector.reciprocal(rs[:], rs[:])
                scale = bp.tile([HN, 1], F32, tag="scale")
                nc.vector.tensor_mul(scale[:], rs[:], w["gam"][:])
                nbias = bp.tile([HN, 1], F32, tag="nbias")
                nc.vector.tensor_mul(nbias[:], mean[:], scale[:])
                nc.vector.scalar_tensor_tensor(out=nbias[:], in0=nbias[:],
                                               scalar=-1.0, in1=w["bet"][:],
                                               op0=AX.mult, op1=AX.add)
                nc.vector.tensor_scalar(out=hTb[:], in0=hTb[:], scalar1=scale[:],
                                        scalar2=nbias[:], op0=AX.mult, op1=AX.add)

            # ---------------- layers ----------------
            for l in range(L):
                # --- table build ---
                with tc.tile_pool(name=f"tb{l}", bufs=2) as tp, \
                     tc.tile_pool(name=f"tq{l}", bufs=1) as tq, \
                     tc.tile_pool(name=f"tp{l}", bufs=2, space="PSUM") as tpp:
                    adT = tq.tile([HN, NL], BF16, tag="adT")
                    nc.vector.memset(ad_nm[:], 0.0)
                    for s in range(G):
                        hls = tp.tile([HN, S], BF16, tag="hls")
                        asl = tp.tile([HN, S], BF16, tag="asl")
                        for ci in range(-(-S // 512)):
                            c0 = ci * 512
                            n = min(512, S - c0)
                            a0 = s * S + c0
                            ph = tpp.tile([HN, 512], F32, tag="ph")
                            nc.tensor.matmul(out=ph[:, :n], lhsT=w["fcw"][:, l, :],
                                             rhs=hTb[:, a0:a0 + n],
                                             start=True, stop=True)
                            nc.vector.tensor_scalar_add(hls[:, c0:c0 + n],
                                                        ph[:, :n],
                                                        w["fcb"][:, l:l + 1])
                            psa = tpp.tile([HN, 512], F32, tag="ph")
                            nc.tensor.matmul(out=psa[:, :n], lhsT=w["wsrc"][:, l, :],
                                             rhs=hls[:, c0:c0 + n],
                                             start=True, stop=True)
                            nc.scalar.copy(out=asl[:, c0:c0 + n], in_=psa[:, :n])
                            psd = tpp.tile([HN, 512], F32, tag="ph")
                            nc.tensor.matmul(out=psd[:, :n], lhsT=w["wdst"][:, l, :],
                                             rhs=hls[:, c0:c0 + n],
                                             start=True, stop=True)
                            nc.vector.tensor_scalar_add(adT[:, a0:a0 + n],
                                                        psd[:, :n],
                                                        w["adb"][:, l:l + 1])
                        # node-major rows for slice s
                        stg = tp.tile([128, cfg.RP, 128], BF16, tag="stg")
                        nc.vector.memset(stg[:], 0.0)
                        for r in range(cfg.RP):
                            c0 = r * 128
                            n = min(128, S - c0)
                            pt1 = tpp.tile([128, HN], BF16, tag="pt")
                            nc.tensor.transpose(out=pt1[:n, :],
                                                in_=asl[:, c0:c0 + n],
                                                identity=ident[:HN, :HN])
                            nc.scalar.copy(out=stg[:n, r, 0:HN], in_=pt1[:n, :])
                            pt2 = tpp.tile([128, HN], BF16, tag="pt")
                            nc.tensor.transpose(out=pt2[:n, :],
                                                in_=hls[:, c0:c0 + n],
                                                identity=ident[:HN, :HN])
                            nc.scalar.copy(out=stg[:n, r, HN:128], in_=pt2[:n, :])
                        nc.sync.dma_start(
                            out=lt[s][:, :].rearrange("(r p) k -> p r k", p=128),
                            in_=stg[:, :, :])
                        nc.gpsimd.collective_compute("AllGather", AX.bypass,
                                                     replica_groups=RG,
                                                     ins=[lt[s][:]],
                                                     outs=[fsh[s][:]])
                    # a_d node-major subtables
                    for sc in range(NSC):
                        c0 = sc * SC
                        n = min(SC, NL - c0)
                        pta = tpp.tile([128, HN], BF16, tag="pt")
                        nc.tensor.transpose(out=pta[:n, :],
                                            in_=adT[:, c0:c0 + n],
                                            identity=ident[:HN, :HN])
                        nc.scalar.copy(out=ad_nm[:n, sc, :], in_=pta[:n, :])
                    nc.sync.dma_start(out=ad_nm[127:128, :, :],
                                      in_=din["urow_b"][l:l + 1, :, :])
                    nc.vector.memset(accum[:], 0.0)

                # --- edge blocks ---
                with tc.tile_pool(name=f"ep{l}", bufs=4) as ep, \
                     tc.tile_pool(name=f"px{l}", bufs=2, space="PSUM") as pxp, \
                     tc.tile_pool(name=f"pa{l}", bufs=4, space="PSUM") as pap:
                    # software-pipelined: loads prefetch 2 ahead, aggregation
                    # of block b-1 interleaves with block b's front half, so
                    # the in-order PE queue never waits on the DVE/ACT chain.
                    state = {"psagg": None}
                    blk = {}

                    def emit_loads(b):
                        g = tile_g[b * BT]
                        gix = ep.tile([128, EB // 16], I16, tag="gix",
                                      name="gix")
                        nc.scalar.dma_start(
                            out=gix[:],
                            in_=din["gidx"][:, b * (EB // 16):(b + 1) * (EB // 16)])
                        od = ep.tile([128, EB], BF16, tag="od", name="od")
                        nc.scalar.dma_start(out=od[:], in_=din["odh"][b, :, :])
                        odt = ep.tile([128, BT, 128], FP8, tag="odt", bufs=5,
                                      name="odt")
                        nc.sync.dma_start(
                            out=odt[:],
                            in_=din["odth"][b, :, :].rearrange(
                                "p (t j) -> p t j", j=128))
                        srcr = ep.tile([128, BT, 128], BF16, tag="srcr",
                                       name="srcr")
                        for hh in range(2):
                            nc.gpsimd.dma_gather(
                                out_ap=srcr[:, hh * (BT // 2):(hh + 1) * (BT // 2), :],
                                in_ap=fsh[g][:].rearrange("c s k -> (c s) k"),
                                idxs_ap=gix[:, hh * (EB // 32):(hh + 1) * (EB // 32)],
                                num_idxs=EB // 2, num_idxs_reg=EB // 2,
                                elem_size=128,
                                single_packet=False, queue_num=(2 * b + hh) % 4)
                        blk[b] = (od, odt, srcr)

                    def emit_front(b):
                        t_base = b * BT
                        od, odt, srcr = blk[b]
                        SUB = cfg.SUB
                        contrib = ep.tile([128, BT, 128], BF16, tag="contrib",
                                          name="contrib")
                        for ww in range(BT // SUB):
                            psx = pxp.tile([128, SUB, HN], F32, tag="psx",
                                           name="psx")
                            for t2 in range(SUB):
                                t = t_base + ww * SUB + t2
                                nc.tensor.matmul(
                                    out=psx[:, t2, :],
                                    lhsT=od[:, (ww * SUB + t2) * TE:(ww * SUB + t2 + 1) * TE],
                                    rhs=ad_nm[:, tile_sc[t], :],
                                    start=True, stop=True)
                            nc.vector.scalar_tensor_tensor(
                                out=contrib[:, ww * SUB:(ww + 1) * SUB, HN:128],
                                in0=psx[:], scalar=1.0,
                                in1=srcr[:, ww * SUB:(ww + 1) * SUB, 0:HN],
                                op0=AX.mult, op1=AX.add)
                        nc.vector.scalar_tensor_tensor(
                            out=contrib[:, :, HN:128],
                            in0=contrib[:, :, HN:128], scalar=cfg.NEG,
                            in1=contrib[:, :, HN:128], op0=AX.mult, op1=AX.max)
                        nc.scalar.activation(out=contrib[:, :, HN:128],
                                             in_=contrib[:, :, HN:128],
                                             func=AF.Exp)
                        nc.vector.tensor_mul(contrib[:, :, 0:HN],
                                             contrib[:, :, HN:128],
                                             srcr[:, :, HN:128])
                        blk[b] = (od, odt, srcr, contrib)

                    def emit_back(b):
                        t_base = b * BT
                        _, odt, _, contrib = blk.pop(b)
                        for t2 in range(BT):
                            t = t_base + t2
                            if tile_st[t]:
                                state["psagg"] = pap.tile([128, 128], F32,
                                                          tag="psagg",
                                                          name="psagg")
                            psagg = state["psagg"]
                            nc.tensor.matmul(out=psagg[:],
                                             lhsT=odt[:, t2, :],
                                             rhs=contrib[:, t2, :],
                                             start=bool(tile_st[t]),
                                             stop=bool(tile_sp[t]))
                            if tile_sp[t]:
                                sc = tile_sc[t]
                                nc.vector.tensor_add(accum[:, sc, :],
                                                     accum[:, sc, :], psagg[:])

                    emit_loads(0)
                    if NB > 1:
                        emit_loads(1)
                    for b in range(NB):
                        if b + 2 < NB:
                            emit_loads(b + 2)
                        emit_front(b)
                        if b >= 1:
                            emit_back(b - 1)
                    emit_back(NB - 1)

                # --- finalize ---
                with tc.tile_pool(name=f"fi{l}", bufs=2) as fp, \
                     tc.tile_pool(name=f"fp{l}", bufs=2, space="PSUM") as fpp:
                    den = fp.tile([128, NSC, HN], F32, tag="den")
                    hnm = fp.tile([128, NSC, HN],
                                  BF16 if l < L - 1 else F32, tag="hnm")
                    for sc in range(NSC):
                        nc.vector.tensor_scalar_add(den[:, sc, :],
                                                    accum[:, sc, HN:128], 1e-16)
                        nc.vector.reciprocal(den[:, sc, :], den[:, sc, :])
                        nc.vector.tensor_mul(hnm[:, sc, :], accum[:, sc, 0:HN],
                                             den[:, sc, :])
                        if l < L - 1:
                            c0 = sc * SC
                            n = min(SC, NL - c0)
                            pth = fpp.tile([HN, 128], BF16, tag="pth")
                            nc.tensor.transpose(out=pth[:], in_=hnm[:, sc, :],
                                                identity=ident[:, :])
                            nc.scalar.copy(out=hTb[:, c0:c0 + n], in_=pth[:, :n])
                    if l == L - 1:
                        nfull = (NSC - 1) * SC       # 12446
                        nc.sync.dma_start(
                            out=out[0:nfull, :].rearrange("(s d) k -> d s k", d=SC),
                            in_=hnm[0:SC, 0:NSC - 1, :])
                        nc.sync.dma_start(
                            out=out[nfull:NL, :],
                            in_=hnm[0:NL - nfull, NSC - 1, :])
    return nc


# --------------------------------------------------------------- entry ------
def kernel(**inputs):
    cfg = Cfg()
    in_maps, meta = host_prepare(cfg, inputs)
    nc = build_nc(cfg, meta)
    nc.compile()
    split_sync_waits(nc)
    res = run_bass_kernel_spmd(nc, in_maps, core_ids=list(range(cfg.C)),
                               trace=bool(int(os.environ.get("GNN_TRACE", "0"))))
    if res.exec_time_ns is not None:
        print(f"HW exec time: {res.exec_time_ns} ns")
    h = np.concatenate([np.asarray(res.results[c]["out"])
                        for c in range(cfg.C)], axis=0)
    return h.reshape(cfg.B, -1, cfg.HN)


# revision 3
# speedup vs baseline: 1.1083x; 1.0190x over previous
"""GNN message-passing encoder (GAT-style) on 8 Trainium2 NeuronCores — v4.

Self-contained: hardcodes the problem shapes (N=100000, E=1600000, HN=64, L=3).

Design (edge-major, bf16, scatterless):
- Core c owns dst nodes [c*NL, (c+1)*NL). Node space re-keyed into G=4 chunks:
  chunk s = slice s of every core (SP=3200 padded rows per core-slice), so each
  chunk's table is completed by ONE sub-AllGather -> gather of chunk g overlaps
  the AllGather of chunk g+1.
- Per layer: table build emits node-major bf16 rows [a_s | hl] per slice,
  4 sub-AllGathers; a_d table kept local node-major (127-node subchunks,
  row 127 = u_l so the ea*u edge term rides the same expand matmul).
- Edges sorted by (chunk, dst-subchunk, dst); tiles of 128 edges never span a
  (chunk, subchunk); blocks of 32 tiles never span a chunk. Per block:
  dma_gather 4096 src rows (256B bf16); one-hot masks od (dst-major, via K=1
  broadcast matmul + is_eq) and odT (edge-major, via is_eq against iota);
  expand matmul a_d+ea*u per tile; alpha = psx + a_s; LeakyReLU; Exp;
  contrib = [ex*hl | ex]; aggregation matmul accumulates [128 dst, 128 ch]
  PSUM across the subchunk's tiles; flushed by DVE add into an SBUF
  accumulator — no scatter, no HBM accum round-trip.
- Finalize: h = num/(den+1e-16) from SBUF, PE-transpose back to channel-major
  for the next layer; final layer DMAs node-major rows out.
"""
import os
import sys

for _p in ("/opt/trn_rl_repo",):
    if _p not in sys.path:
        sys.path.insert(0, _p)

import numpy as np
import ml_dtypes

import concourse.bass as bass
import concourse.mybir as mybir
import concourse.tile as tile
from concourse import bacc
from concourse import library_config as libcfg
from concourse.bass_utils import run_bass_kernel_spmd

F32 = mybir.dt.float32
BF16 = mybir.dt.bfloat16
I16 = mybir.dt.int16
AX = mybir.AluOpType
AF = mybir.ActivationFunctionType
BF16NP = ml_dtypes.bfloat16
FP8 = mybir.dt.float8e4
ODT_NP = ml_dtypes.float8_e4m3


class Cfg:
    def __init__(self, N=100000, E=1600000, BT=32, B=10):
        self.N, self.E, self.C, self.HN, self.IN_N, self.L, self.B = \
            N, E, 8, 64, 3, 3, B
        self.NEG, self.EPS = 0.2, 1e-5
        self.NL = self.N // self.C          # 12500 local dst nodes
        self.G = 4                          # src chunks
        self.S = self.NL // self.G          # 3125 real rows per slice
        self.SP = -(-self.S // 128) * 128   # 3200 padded slice rows
        self.CH = self.C * self.SP          # 25600 chunk rows
        self.RP = self.SP // 128            # staging row-groups per slice
        self.SC = 127                       # dst nodes per subchunk
        self.NSC = -(-self.NL // self.SC)   # 99 subchunks
        self.TE = 128
        self.BT = BT
        self.SUB = min(8, BT)
        self.EB = self.TE * BT              # 4096 edges per block
        assert self.NL == self.G * self.S and self.CH <= 32768


# ------------------------------------------------------------- host prep ----
def _pack_idx16(vals):
    v = np.asarray(vals, dtype=np.int16)
    assert v.size % 16 == 0
    return np.tile(v.reshape(-1, 16).T, (8, 1)).copy()


def prep_edges(cfg, src, dst, ea):
    """Per-core edge partition grouped by (g, sc); returns per-core dict of
    (g, sc) -> (srcidx array, code array, ea array)."""
    per_core = []
    for c in range(cfg.C):
        m = (dst >= c * cfg.NL) & (dst < (c + 1) * cfg.NL)
        s_c, d_c, e_c = src[m], dst[m] - c * cfg.NL, ea[m]
        g = (s_c % cfg.NL) // cfg.S
        sidx = (s_c // cfg.NL) * cfg.SP + (s_c % cfg.S)
        sc = d_c // cfg.SC
        code = d_c % cfg.SC
        order = np.lexsort((d_c, sc, g))
        per_core.append(dict(g=g[order], sidx=sidx[order], sc=sc[order],
                             code=code[order], ea=e_c[order]))
    return per_core


def build_tiles(cfg, per_core):
    """Global tile layout (shared across cores) + per-core packed streams."""
    C, G, NSC, TE, BT = cfg.C, cfg.G, cfg.NSC, cfg.TE, cfg.BT
    # edge counts per (core, g, sc)
    cnt = np.zeros((C, G, NSC), np.int64)
    for c in range(C):
        pc = per_core[c]
        np.add.at(cnt[c], (pc["g"], pc["sc"]), 1)
    T = np.maximum(cnt, 0).max(axis=0)          # [G, NSC] max count
    T = -(-T // TE)                              # tiles per (g, sc)
    # pad each g to a multiple of BT (extend last subchunk's tile count)
    for g in range(G):
        rem = (-T[g].sum()) % BT
        T[g, NSC - 1] += rem
    NBg = [int(T[g].sum()) // BT for g in range(G)]
    TT = int(T.sum())
    NB = sum(NBg)
    assert TT == NB * BT

    # shared per-tile metadata
    tile_g = np.zeros(TT, np.int32)
    tile_sc = np.zeros(TT, np.int32)
    tile_st = np.zeros(TT, bool)
    tile_sp = np.zeros(TT, bool)
    t = 0
    for g in range(G):
        for sc in range(NSC):
            n = int(T[g, sc])
            if n == 0:
                continue
            tile_g[t:t + n] = g
            tile_sc[t:t + n] = sc
            tile_st[t] = True
            tile_sp[t + n - 1] = True
            t += n
    assert t == TT

    # per-core streams
    streams = []
    d127 = np.arange(127)
    for c in range(C):
        pc = per_core[c]
        gidx = np.zeros(TT * TE, np.int16)
        codeF = np.full(TT * TE, -1, np.int32)
        earow = np.zeros(TT * TE, np.float32)
        # group start offsets in the sorted stream
        t0 = 0
        p = 0  # pointer into sorted edge stream
        for g in range(G):
            for sc in range(NSC):
                n_edges = int(cnt[c, g, sc])
                ntiles = int(T[g, sc])
                base = t0 * TE
                sl = slice(p, p + n_edges)
                gidx[base:base + n_edges] = pc["sidx"][sl].astype(np.int16)
                codeF[base:base + n_edges] = pc["code"][sl]
                earow[base:base + n_edges] = pc["ea"][sl]
                p += n_edges
                t0 += ntiles
        assert p == len(pc["g"])
        # od[d, e] one-hot (dst-major) with ea on row 127; odT[p, t, d]
        od = np.zeros((128, TT * TE), BF16NP)
        od[:127, :] = (codeF[None, :] == d127[:, None]).astype(BF16NP)
        od[127, :] = earow.astype(BF16NP)
        odT = (codeF.reshape(TT, TE).T[:, :, None]
               == d127[None, None, :]).astype(ODT_NP)   # [p, TT, 127]
        odTf = np.zeros((128, TT, 128), ODT_NP)
        odTf[:, :, :127] = odT
        streams.append(dict(
            gidx=_pack_idx16(gidx),
            odh=od.reshape(128, NB, cfg.EB).transpose(1, 0, 2).copy(),
            odth=odTf.reshape(128, NB, BT * 128).transpose(1, 0, 2).copy(),
        ))
    meta = dict(NB=NB, NBg=NBg, TT=TT, tile_g=tile_g, tile_sc=tile_sc,
                tile_st=tile_st, tile_sp=tile_sp)
    return meta, streams


def host_prepare(cfg, inputs):
    ei = np.asarray(inputs["edge_index"])
    src, dst = ei[0].astype(np.int64), ei[1].astype(np.int64)
    ea = np.asarray(inputs["edge_attr"], np.float32)[:, 0]
    per_core = prep_edges(cfg, src, dst, ea)
    meta, streams = build_tiles(cfg, per_core)

    HN, L = cfg.HN, cfg.L
    fnw = np.asarray(inputs["fc_node_w"], np.float32)
    fnb = np.asarray(inputs["fc_node_b"], np.float32)
    few = np.asarray(inputs["fc_edge_w"], np.float32)
    feb = np.asarray(inputs["fc_edge_b"], np.float32)
    gam = np.asarray(inputs["bn_gamma"], np.float32)
    bet = np.asarray(inputs["bn_beta"], np.float32)
    fcw = np.asarray(inputs["fc_w"], np.float32)
    fcb = np.asarray(inputs["fc_b"], np.float32)
    aw = np.asarray(inputs["attn_w"], np.float32)
    ab = np.asarray(inputs["attn_b"], np.float32)
    wdst, wsrc, we = aw[:, :HN, :], aw[:, HN:2 * HN, :], aw[:, 2 * HN:, :]
    u = np.stack([few[0] @ we[l] for l in range(L)])          # [L, 64]
    adb = np.stack([ab[l] + feb @ we[l] for l in range(L)])   # [L, 64]
    x = np.asarray(inputs["x"], np.float32)

    shared = dict(
        fnw=fnw, fnb=fnb.reshape(-1, 1), gam=gam.reshape(-1, 1),
        bet=bet.reshape(-1, 1),
        fcw=np.ascontiguousarray(fcw.transpose(1, 0, 2)).astype(BF16NP),
        wsrc=np.ascontiguousarray(wsrc.transpose(1, 0, 2)).astype(BF16NP),
        wdst=np.ascontiguousarray(wdst.transpose(1, 0, 2)).astype(BF16NP),
        fcb=np.ascontiguousarray(fcb.T), adb=np.ascontiguousarray(adb.T),
        urow=u.astype(BF16NP),                                 # [L, 64]
        urow_b=np.broadcast_to(u.astype(BF16NP)[:, None, :],
                               (L, cfg.NSC, HN)).copy(),
        identb=np.eye(128, dtype=BF16NP),
    )
    in_maps = []
    for c in range(cfg.C):
        im = dict(shared)
        im.update(streams[c])
        im["xT"] = np.ascontiguousarray(
            x[c * cfg.NL:(c + 1) * cfg.NL].T).astype(BF16NP)
        in_maps.append(im)
    return in_maps, meta


# --------------------------------------------------------------- builder ----
def split_sync_waits(nc, max_waits=1):
    for f in nc.m.functions:
        for bb in f.blocks:
            old = bb.instructions
            if not any(i.sync_info and i.sync_info.on_wait
                       and len(i.sync_info.on_wait) > max_waits for i in old):
                continue
            new = []
            for ins in old:
                si = ins.sync_info
                if si is not None and si.on_wait and len(si.on_wait) > max_waits:
                    waits = list(si.on_wait)
                    extra, keep = waits[:-max_waits], waits[-max_waits:]
                    for j, w in enumerate(extra):
                        nop = mybir.InstNoOp(name=f"{ins.name}-wc{j}", ins=[], outs=[])
                        nop.engine = ins.engine
                        nop.sync_info = mybir.SyncInfo(on_wait=[w], on_update=[])
                        new.append(nop)
                    si.on_wait = keep
                new.append(ins)
            bb.instructions = new
    return nc


def build_nc(cfg, meta):
    NL, G, S, SP, CH = cfg.NL, cfg.G, cfg.S, cfg.SP, cfg.CH
    SC, NSC, TE, BT, EB = cfg.SC, cfg.NSC, cfg.TE, cfg.BT, cfg.EB
    HN, L, C, N = cfg.HN, cfg.L, cfg.C, cfg.N
    NB, NBg, TT = meta["NB"], meta["NBg"], meta["TT"]
    tile_g, tile_sc = meta["tile_g"], meta["tile_sc"]
    tile_st, tile_sp = meta["tile_st"], meta["tile_sp"]
    RG = [list(range(C))]

    nc = bacc.Bacc(None, target_bir_lowering=False, num_swdge_queues=4)
    din = {}

    def ext(name, shape, dt=F32):
        din[name] = nc.dram_tensor(name, shape, dt, kind="ExternalInput")

    ext("xT", [cfg.IN_N, NL], BF16)
    ext("gidx", [128, TT * TE // 16], I16)
    ext("odh", [NB, 128, EB], BF16)
    ext("odth", [NB, 128, BT * 128], FP8)
    ext("fnw", [cfg.IN_N, HN]); ext("fnb", [HN, 1])
    ext("gam", [HN, 1]); ext("bet", [HN, 1])
    ext("fcw", [HN, L, HN], BF16); ext("fcb", [HN, L])
    ext("wsrc", [HN, L, HN], BF16); ext("wdst", [HN, L, HN], BF16)
    ext("adb", [HN, L])
    ext("urow", [L, HN], BF16)
    ext("urow_b", [L, NSC, HN], BF16)
    ext("identb", [128, 128], BF16)

    out = nc.dram_tensor("out", [NL, HN], F32, kind="ExternalOutput")
    lt = [nc.dram_tensor(f"lt{s}", [SP, 128], BF16) for s in range(G)]
    fsh = [nc.dram_tensor(f"fsh{s}", [C, SP, 128], BF16, addr_space="Shared")
           for s in range(G)]
    bn_loc = nc.dram_tensor("bn_loc", [HN, 2], F32)
    bn_sh = nc.dram_tensor("bn_sh", [HN, 2], F32, addr_space="Shared")

    with tile.TileContext(nc) as tc:
        with tc.tile_pool(name="persist", bufs=1) as pp:
            nc.gpsimd.load_library(libcfg.mlp)
            w = {}
            for nm, shp, dt in (
                ("fnw", [cfg.IN_N, HN], F32), ("fnb", [HN, 1], F32),
                ("gam", [HN, 1], F32), ("bet", [HN, 1], F32),
                ("fcw", [HN, L, HN], BF16), ("fcb", [HN, L], F32),
                ("wsrc", [HN, L, HN], BF16), ("wdst", [HN, L, HN], BF16),
                ("adb", [HN, L], F32), ("urow", [L, HN], BF16),
                ("identb", [128, 128], BF16),
            ):
                w[nm] = pp.tile(shp, dt, tag=nm, name=nm)
                nc.sync.dma_start(out=w[nm][:], in_=din[nm][:])
            ident = w["identb"]
            hTb = pp.tile([HN, NL], BF16, tag="hTb", name="hTb")
            ad_nm = pp.tile([128, NSC, HN], BF16, tag="ad_nm", name="ad_nm")
            accum = pp.tile([128, NSC, 128], F32, tag="accum", name="accum")

            # ---------------- input proj + BN (f32) ----------------
            with tc.tile_pool(name="bn", bufs=2) as bp, \
                 tc.tile_pool(name="bnsq", bufs=1) as bq, \
                 tc.tile_pool(name="bnp", bufs=2, space="PSUM") as bpp:
                xt = bq.tile([cfg.IN_N, NL], BF16, tag="xt")
                nc.sync.dma_start(out=xt[:], in_=din["xT"][:])
                fnwb = bp.tile([cfg.IN_N, HN], BF16, tag="fnwb")
                nc.vector.tensor_copy(fnwb[:], w["fnw"][:])
                for ci in range(-(-NL // 512)):
                    c0 = ci * 512
                    n = min(512, NL - c0)
                    ps = bpp.tile([HN, 512], F32, tag="ps")
                    nc.tensor.matmul(out=ps[:, :n], lhsT=fnwb[:],
                                     rhs=xt[:, c0:c0 + n], start=True, stop=True)
                    nc.vector.tensor_scalar_add(hTb[:, c0:c0 + n], ps[:, :n],
                                                w["fnb"][:])
                st = bp.tile([HN, 2], F32, tag="st")
                stp = bp.tile([HN, cfg.G], F32, tag="stp")
                sqc = bq.tile([HN, S], F32, tag="sqc")
                nc.vector.reduce_sum(st[:, 0:1], hTb[:], axis=mybir.AxisListType.X)
                for j in range(cfg.G):
                    nc.vector.scalar_tensor_tensor(
                        out=sqc[:], in0=hTb[:, j * S:(j + 1) * S], scalar=1.0,
                        in1=hTb[:, j * S:(j + 1) * S], op0=AX.mult, op1=AX.mult,
                        accum_out=stp[:, j:j + 1])
                nc.vector.reduce_sum(st[:, 1:2], stp[:], axis=mybir.AxisListType.X)
                nc.sync.dma_start(out=bn_loc[:], in_=st[:])
                nc.gpsimd.collective_compute("AllReduce", AX.add,
                                             replica_groups=RG,
                                             ins=[bn_loc[:]], outs=[bn_sh[:]])
                sg = bp.tile([HN, 2], F32, tag="sg")
                nc.sync.dma_start(out=sg[:], in_=bn_sh[:])
                mean = bp.tile([HN, 1], F32, tag="mean")
                var = bp.tile([HN, 1], F32, tag="var")
                nc.vector.tensor_scalar_mul(mean[:], sg[:, 0:1], 1.0 / N)
                nc.vector.tensor_scalar_mul(var[:], sg[:, 1:2], 1.0 / N)
                msq = bp.tile([HN, 1], F32, tag="msq")
                nc.vector.tensor_mul(msq[:], mean[:], mean[:])
                nc.vector.tensor_sub(var[:], var[:], msq[:])
                nc.vector.tensor_scalar_add(var[:], var[:], cfg.EPS)
                rs = bp.tile([HN, 1], F32, tag="rs")
                nc.scalar.activation(out=rs[:], in_=var[:], func=AF.Sqrt)
                nc.vector.reciprocal(rs[:], rs[:])
                scale = bp.tile([HN, 1], F32, tag="scale")
                nc.vector.tensor_mul(scale[:], rs[:], w["gam"][:])
                nbias = bp.tile([HN, 1], F32, tag="nbias")
                nc.vector.tensor_mul(nbias[:], mean[:], scale[:])
                nc.vector.scalar_tensor_tensor(out=nbias[:], in0=nbias[:],
                                               scalar=-1.0, in1=w["bet"][:],
                                               op0=AX.mult, op1=AX.add)
                nc.vector.tensor_scalar(out=hTb[:], in0=hTb[:], scalar1=scale[:],
                                        scalar2=nbias[:], op0=AX.mult, op1=AX.add)

            # ---------------- layers ----------------
            for l in range(L):
                # --- table build ---
                with tc.tile_pool(name=f"tb{l}", bufs=2) as tp, \
                     tc.tile_pool(name=f"tq{l}", bufs=1) as tq, \
                     tc.tile_pool(name=f"tp{l}", bufs=2, space="PSUM") as tpp:
                    adT = tq.tile([HN, NL], BF16, tag="adT")
                    nc.vector.memset(ad_nm[:], 0.0)
                    for s in range(G):
                        hls = tp.tile([HN, S], BF16, tag="hls")
                        asl = tp.tile([HN, S], BF16, tag="asl")
                        for ci in range(-(-S // 512)):
                            c0 = ci * 512
                            n = min(512, S - c0)
                            a0 = s * S + c0
                            ph = tpp.tile([HN, 512], F32, tag="ph")
                            nc.tensor.matmul(out=ph[:, :n], lhsT=w["fcw"][:, l, :],
                                             rhs=hTb[:, a0:a0 + n],
                                             start=True, stop=True)
                            nc.vector.tensor_scalar_add(hls[:, c0:c0 + n],
                                                        ph[:, :n],
                                                        w["fcb"][:, l:l + 1])
                            psa = tpp.tile([HN, 512], F32, tag="ph")
                            nc.tensor.matmul(out=psa[:, :n], lhsT=w["wsrc"][:, l, :],
                                             rhs=hls[:, c0:c0 + n],
                                             start=True, stop=True)
                            nc.scalar.copy(out=asl[:, c0:c0 + n], in_=psa[:, :n])
                            psd = tpp.tile([HN, 512], F32, tag="ph")
                            nc.tensor.matmul(out=psd[:, :n], lhsT=w["wdst"][:, l, :],
                                             rhs=hls[:, c0:c0 + n],
                                             start=True, stop=True)
                            nc.vector.tensor_scalar_add(adT[:, a0:a0 + n],
                                                        psd[:, :n],
                                                        w["adb"][:, l:l + 1])
                        # node-major rows for slice s
                        stg = tp.tile([128, cfg.RP, 128], BF16, tag="stg")
                        nc.vector.memset(stg[:], 0.0)
                        for r in range(cfg.RP):
                            c0 = r * 128
                            n = min(128, S - c0)
                            pt1 = tpp.tile([128, HN], BF16, tag="pt")
                            nc.tensor.transpose(out=pt1[:n, :],
                                                in_=asl[:, c0:c0 + n],
                                                identity=ident[:HN, :HN])
                            nc.scalar.copy(out=stg[:n, r, 0:HN], in_=pt1[:n, :])
                            pt2 = tpp.tile([128, HN], BF16, tag="pt")
                            nc.tensor.transpose(out=pt2[:n, :],
                                                in_=hls[:, c0:c0 + n],
                                                identity=ident[:HN, :HN])
                            nc.scalar.copy(out=stg[:n, r, HN:128], in_=pt2[:n, :])
                        nc.sync.dma_start(
                            out=lt[s][:, :].rearrange("(r p) k -> p r k", p=128),
                            in_=stg[:, :, :])
                        nc.gpsimd.collective_compute("AllGather", AX.bypass,
                                                     replica_groups=RG,
                                                     ins=[lt[s][:]],
                                                     outs=[fsh[s][:]])
                    # a_d node-major subtables
                    for sc in range(NSC):
                        c0 = sc * SC
                        n = min(SC, NL - c0)
                        pta = tpp.tile([128, HN], BF16, tag="pt")
                        nc.tensor.transpose(out=pta[:n, :],
                                            in_=adT[:, c0:c0 + n],
                                            identity=ident[:HN, :HN])
                        nc.scalar.copy(out=ad_nm[:n, sc, :], in_=pta[:n, :])
                    nc.sync.dma_start(out=ad_nm[127:128, :, :],
                                      in_=din["urow_b"][l:l + 1, :, :])
                    nc.vector.memset(accum[:], 0.0)

                # --- edge blocks ---
                with tc.tile_pool(name=f"ep{l}", bufs=3) as ep, \
                     tc.tile_pool(name=f"px{l}", bufs=2, space="PSUM") as pxp, \
                     tc.tile_pool(name=f"pa{l}", bufs=4, space="PSUM") as pap:
                    psagg = None
                    for b in range(NB):
                        g = tile_g[b * BT]
                        t_base = b * BT
                        gix = ep.tile([128, EB // 16], I16, tag="gix")
                        nc.scalar.dma_start(
                            out=gix[:],
                            in_=din["gidx"][:, b * (EB // 16):(b + 1) * (EB // 16)])
                        od = ep.tile([128, EB], BF16, tag="od")
                        nc.scalar.dma_start(out=od[:], in_=din["odh"][b, :, :])
                        odt = ep.tile([128, BT, 128], FP8, tag="odt")
                        nc.sync.dma_start(
                            out=odt[:],
                            in_=din["odth"][b, :, :].rearrange(
                                "p (t j) -> p t j", j=128))
                        srcr = ep.tile([128, BT, 128], BF16, tag="srcr")
                        for hh in range(2):
                            nc.gpsimd.dma_gather(
                                out_ap=srcr[:, hh * (BT // 2):(hh + 1) * (BT // 2), :],
                                in_ap=fsh[g][:].rearrange("c s k -> (c s) k"),
                                idxs_ap=gix[:, hh * (EB // 32):(hh + 1) * (EB // 32)],
                                num_idxs=EB // 2, num_idxs_reg=EB // 2,
                                elem_size=128,
                                single_packet=False, queue_num=(2 * b + hh) % 4)
                        # expand + alpha
                        SUB = cfg.SUB
                        contrib = ep.tile([128, BT, 128], BF16, tag="contrib")
                        for ww in range(BT // SUB):
                            psx = pxp.tile([128, SUB, HN], F32, tag="psx")
                            for t2 in range(SUB):
                                t = t_base + ww * SUB + t2
                                nc.tensor.matmul(
                                    out=psx[:, t2, :],
                                    lhsT=od[:, (ww * SUB + t2) * TE:(ww * SUB + t2 + 1) * TE],
                                    rhs=ad_nm[:, tile_sc[t], :],
                                    start=True, stop=True)
                            nc.vector.scalar_tensor_tensor(
                                out=contrib[:, ww * SUB:(ww + 1) * SUB, HN:128],
                                in0=psx[:], scalar=1.0,
                                in1=srcr[:, ww * SUB:(ww + 1) * SUB, 0:HN],
                                op0=AX.mult, op1=AX.add)
                        nc.vector.scalar_tensor_tensor(
                            out=contrib[:, :, HN:128],
                            in0=contrib[:, :, HN:128], scalar=cfg.NEG,
                            in1=contrib[:, :, HN:128], op0=AX.mult, op1=AX.max)
                        nc.scalar.activation(out=contrib[:, :, HN:128],
                                             in_=contrib[:, :, HN:128],
                                             func=AF.Exp)
                        nc.vector.tensor_mul(contrib[:, :, 0:HN],
                                             contrib[:, :, HN:128],
                                             srcr[:, :, HN:128])
                        # aggregation
                        for t2 in range(BT):
                            t = t_base + t2
                            if tile_st[t]:
                                psagg = pap.tile([128, 128], F32, tag="psagg")
                            nc.tensor.matmul(out=psagg[:],
                                             lhsT=odt[:, t2, :],
                                             rhs=contrib[:, t2, :],
                                             start=bool(tile_st[t]),
                                             stop=bool(tile_sp[t]))
                            if tile_sp[t]:
                                sc = tile_sc[t]
                                nc.vector.tensor_add(accum[:, sc, :],
                                                     accum[:, sc, :], psagg[:])

                # --- finalize ---
                with tc.tile_pool(name=f"fi{l}", bufs=2) as fp, \
                     tc.tile_pool(name=f"fp{l}", bufs=2, space="PSUM") as fpp:
                    den = fp.tile([128, NSC, HN], F32, tag="den")
                    nc.vector.tensor_scalar_add(den[:], accum[:, :, HN:128], 1e-16)
                    nc.vector.reciprocal(den[:], den[:])
                    if l < L - 1:
                        hnm = fp.tile([128, NSC, HN], BF16, tag="hnm")
                        nc.vector.tensor_mul(hnm[:], accum[:, :, 0:HN], den[:])
                        for sc in range(NSC):
                            c0 = sc * SC
                            n = min(SC, NL - c0)
                            pth = fpp.tile([HN, 128], BF16, tag="pth")
                            nc.tensor.transpose(out=pth[:], in_=hnm[:, sc, :],
                                                identity=ident[:, :])
                            nc.scalar.copy(out=hTb[:, c0:c0 + n], in_=pth[:, :n])
                    else:
                        hno = fp.tile([128, NSC, HN], F32, tag="hno")
                        nc.vector.tensor_mul(hno[:], accum[:, :, 0:HN], den[:])
                        nfull = (NSC - 1) * SC       # 12446
                        nc.sync.dma_start(
                            out=out[0:nfull, :].rearrange("(s d) k -> d s k", d=SC),
                            in_=hno[0:SC, 0:NSC - 1, :])
                        nc.sync.dma_start(
                            out=out[nfull:NL, :],
                            in_=hno[0:NL - nfull, NSC - 1, :])
    return nc


# --------------------------------------------------------------- entry ------
def kernel(**inputs):
    cfg = Cfg()
    in_maps, meta = host_prepare(cfg, inputs)
    nc = build_nc(cfg, meta)
    nc.compile()
    split_sync_waits(nc)
    res = run_bass_kernel_spmd(nc, in_maps, core_ids=list(range(cfg.C)),
                               trace=bool(int(os.environ.get("GNN_TRACE", "0"))))
    if res.exec_time_ns is not None:
        print(f"HW exec time: {res.exec_time_ns} ns")
    h = np.concatenate([np.asarray(res.results[c]["out"])
                        for c in range(cfg.C)], axis=0)
    return h.reshape(cfg.B, -1, cfg.HN)


# revision 4
# speedup vs baseline: 1.1683x; 1.0542x over previous
"""GNN message-passing encoder (GAT-style) on 8 Trainium2 NeuronCores — v4.

Self-contained: hardcodes the problem shapes (N=100000, E=1600000, HN=64, L=3).

Design (edge-major, bf16, scatterless):
- Core c owns dst nodes [c*NL, (c+1)*NL). Node space re-keyed into G=4 chunks:
  chunk s = slice s of every core (SP=3200 padded rows per core-slice), so each
  chunk's table is completed by ONE sub-AllGather -> gather of chunk g overlaps
  the AllGather of chunk g+1.
- Per layer: table build emits node-major bf16 rows [a_s | hl] per slice,
  4 sub-AllGathers; a_d table kept local node-major (127-node subchunks,
  row 127 = u_l so the ea*u edge term rides the same expand matmul).
- Edges sorted by (chunk, dst-subchunk, dst); tiles of 128 edges never span a
  (chunk, subchunk); blocks of 32 tiles never span a chunk. Per block:
  dma_gather 4096 src rows (256B bf16); one-hot masks od (dst-major, via K=1
  broadcast matmul + is_eq) and odT (edge-major, via is_eq against iota);
  expand matmul a_d+ea*u per tile; alpha = psx + a_s; LeakyReLU; Exp;
  contrib = [ex*hl | ex]; aggregation matmul accumulates [128 dst, 128 ch]
  PSUM across the subchunk's tiles; flushed by DVE add into an SBUF
  accumulator — no scatter, no HBM accum round-trip.
- Finalize: h = num/(den+1e-16) from SBUF, PE-transpose back to channel-major
  for the next layer; final layer DMAs node-major rows out.
"""
import os
import sys

for _p in ("/opt/trn_rl_repo",):
    if _p not in sys.path:
        sys.path.insert(0, _p)

import numpy as np
import ml_dtypes

import concourse.bass as bass
import concourse.mybir as mybir
import concourse.tile as tile
from concourse import bacc
from concourse import library_config as libcfg
from concourse.bass_utils import run_bass_kernel_spmd

F32 = mybir.dt.float32
BF16 = mybir.dt.bfloat16
I16 = mybir.dt.int16
AX = mybir.AluOpType
AF = mybir.ActivationFunctionType
BF16NP = ml_dtypes.bfloat16
FP8 = mybir.dt.float8e4
ODT_NP = ml_dtypes.float8_e4m3


class Cfg:
    def __init__(self, N=100000, E=1600000, BT=32, B=10):
        self.N, self.E, self.C, self.HN, self.IN_N, self.L, self.B = \
            N, E, 8, 64, 3, 3, B
        self.NEG, self.EPS = 0.2, 1e-5
        self.NL = self.N // self.C          # 12500 local dst nodes
        self.G = 4                          # src chunks
        self.S = self.NL // self.G          # 3125 real rows per slice
        self.SP = -(-self.S // 128) * 128   # 3200 padded slice rows
        self.CH = self.C * self.SP          # 25600 chunk rows
        self.RP = self.SP // 128            # staging row-groups per slice
        self.SC = 127                       # dst nodes per subchunk
        self.NSC = -(-self.NL // self.SC)   # 99 subchunks
        self.TE = 128
        self.BT = BT
        self.SUB = min(8, BT)
        self.EB = self.TE * BT              # 4096 edges per block
        assert self.NL == self.G * self.S and self.CH <= 32768


# ------------------------------------------------------------- host prep ----
def _pack_idx16(vals):
    v = np.asarray(vals, dtype=np.int16)
    assert v.size % 16 == 0
    return np.tile(v.reshape(-1, 16).T, (8, 1)).copy()


def prep_edges(cfg, src, dst, ea):
    """Per-core edge partition grouped by (g, sc); returns per-core dict of
    (g, sc) -> (srcidx array, code array, ea array)."""
    per_core = []
    for c in range(cfg.C):
        m = (dst >= c * cfg.NL) & (dst < (c + 1) * cfg.NL)
        s_c, d_c, e_c = src[m], dst[m] - c * cfg.NL, ea[m]
        g = (s_c % cfg.NL) // cfg.S
        sidx = (s_c // cfg.NL) * cfg.SP + (s_c % cfg.S)
        sc = d_c // cfg.SC
        code = d_c % cfg.SC
        order = np.lexsort((d_c, sc, g))
        per_core.append(dict(g=g[order], sidx=sidx[order], sc=sc[order],
                             code=code[order], ea=e_c[order]))
    return per_core


def build_tiles(cfg, per_core):
    """Global tile layout (shared across cores) + per-core packed streams."""
    C, G, NSC, TE, BT = cfg.C, cfg.G, cfg.NSC, cfg.TE, cfg.BT
    # edge counts per (core, g, sc)
    cnt = np.zeros((C, G, NSC), np.int64)
    for c in range(C):
        pc = per_core[c]
        np.add.at(cnt[c], (pc["g"], pc["sc"]), 1)
    T = np.maximum(cnt, 0).max(axis=0)          # [G, NSC] max count
    T = -(-T // TE)                              # tiles per (g, sc)
    # pad each g to a multiple of BT (extend last subchunk's tile count)
    for g in range(G):
        rem = (-T[g].sum()) % BT
        T[g, NSC - 1] += rem
    NBg = [int(T[g].sum()) // BT for g in range(G)]
    TT = int(T.sum())
    NB = sum(NBg)
    assert TT == NB * BT

    # shared per-tile metadata
    tile_g = np.zeros(TT, np.int32)
    tile_sc = np.zeros(TT, np.int32)
    tile_st = np.zeros(TT, bool)
    tile_sp = np.zeros(TT, bool)
    t = 0
    for g in range(G):
        for sc in range(NSC):
            n = int(T[g, sc])
            if n == 0:
                continue
            tile_g[t:t + n] = g
            tile_sc[t:t + n] = sc
            tile_st[t] = True
            tile_sp[t + n - 1] = True
            t += n
    assert t == TT

    # per-core streams
    streams = []
    d127 = np.arange(127)
    for c in range(C):
        pc = per_core[c]
        gidx = np.zeros(TT * TE, np.int16)
        codeF = np.full(TT * TE, -1, np.int32)
        earow = np.zeros(TT * TE, np.float32)
        # group start offsets in the sorted stream
        t0 = 0
        p = 0  # pointer into sorted edge stream
        for g in range(G):
            for sc in range(NSC):
                n_edges = int(cnt[c, g, sc])
                ntiles = int(T[g, sc])
                base = t0 * TE
                sl = slice(p, p + n_edges)
                gidx[base:base + n_edges] = pc["sidx"][sl].astype(np.int16)
                codeF[base:base + n_edges] = pc["code"][sl]
                earow[base:base + n_edges] = pc["ea"][sl]
                p += n_edges
                t0 += ntiles
        assert p == len(pc["g"])
        # od[d, e] one-hot (dst-major) with ea on row 127; odT[p, t, d]
        od = np.zeros((128, TT * TE), BF16NP)
        od[:127, :] = (codeF[None, :] == d127[:, None]).astype(BF16NP)
        od[127, :] = earow.astype(BF16NP)
        odT = (codeF.reshape(TT, TE).T[:, :, None]
               == d127[None, None, :]).astype(ODT_NP)   # [p, TT, 127]
        odTf = np.zeros((128, TT, 128), ODT_NP)
        odTf[:, :, :127] = odT
        streams.append(dict(
            gidx=_pack_idx16(gidx),
            odh=od.reshape(128, NB, cfg.EB).transpose(1, 0, 2).copy(),
            odth=odTf.reshape(128, NB, BT * 128).transpose(1, 0, 2).copy(),
        ))
    meta = dict(NB=NB, NBg=NBg, TT=TT, tile_g=tile_g, tile_sc=tile_sc,
                tile_st=tile_st, tile_sp=tile_sp)
    return meta, streams


def host_prepare(cfg, inputs):
    ei = np.asarray(inputs["edge_index"])
    src, dst = ei[0].astype(np.int64), ei[1].astype(np.int64)
    ea = np.asarray(inputs["edge_attr"], np.float32)[:, 0]
    per_core = prep_edges(cfg, src, dst, ea)
    meta, streams = build_tiles(cfg, per_core)

    HN, L = cfg.HN, cfg.L
    fnw = np.asarray(inputs["fc_node_w"], np.float32)
    fnb = np.asarray(inputs["fc_node_b"], np.float32)
    few = np.asarray(inputs["fc_edge_w"], np.float32)
    feb = np.asarray(inputs["fc_edge_b"], np.float32)
    gam = np.asarray(inputs["bn_gamma"], np.float32)
    bet = np.asarray(inputs["bn_beta"], np.float32)
    fcw = np.asarray(inputs["fc_w"], np.float32)
    fcb = np.asarray(inputs["fc_b"], np.float32)
    aw = np.asarray(inputs["attn_w"], np.float32)
    ab = np.asarray(inputs["attn_b"], np.float32)
    wdst, wsrc, we = aw[:, :HN, :], aw[:, HN:2 * HN, :], aw[:, 2 * HN:, :]
    u = np.stack([few[0] @ we[l] for l in range(L)])          # [L, 64]
    adb = np.stack([ab[l] + feb @ we[l] for l in range(L)])   # [L, 64]
    x = np.asarray(inputs["x"], np.float32)

    shared = dict(
        fnw=fnw, fnb=fnb.reshape(-1, 1), gam=gam.reshape(-1, 1),
        bet=bet.reshape(-1, 1),
        fcw=np.ascontiguousarray(fcw.transpose(1, 0, 2)).astype(BF16NP),
        wsrc=np.ascontiguousarray(wsrc.transpose(1, 0, 2)).astype(BF16NP),
        wdst=np.ascontiguousarray(wdst.transpose(1, 0, 2)).astype(BF16NP),
        fcb=np.ascontiguousarray(fcb.T), adb=np.ascontiguousarray(adb.T),
        urow=u.astype(BF16NP),                                 # [L, 64]
        urow_b=np.broadcast_to(u.astype(BF16NP)[:, None, :],
                               (L, cfg.NSC, HN)).copy(),
        identb=np.eye(128, dtype=BF16NP),
    )
    in_maps = []
    for c in range(cfg.C):
        im = dict(shared)
        im.update(streams[c])
        im["xT"] = np.ascontiguousarray(
            x[c * cfg.NL:(c + 1) * cfg.NL].T).astype(BF16NP)
        in_maps.append(im)
    return in_maps, meta


# --------------------------------------------------------------- builder ----
def split_sync_waits(nc, max_waits=1):
    for f in nc.m.functions:
        for bb in f.blocks:
            old = bb.instructions
            if not any(i.sync_info and i.sync_info.on_wait
                       and len(i.sync_info.on_wait) > max_waits for i in old):
                continue
            new = []
            for ins in old:
                si = ins.sync_info
                if si is not None and si.on_wait and len(si.on_wait) > max_waits:
                    waits = list(si.on_wait)
                    extra, keep = waits[:-max_waits], waits[-max_waits:]
                    for j, w in enumerate(extra):
                        nop = mybir.InstNoOp(name=f"{ins.name}-wc{j}", ins=[], outs=[])
                        nop.engine = ins.engine
                        nop.sync_info = mybir.SyncInfo(on_wait=[w], on_update=[])
                        new.append(nop)
                    si.on_wait = keep
                new.append(ins)
            bb.instructions = new
    return nc


def build_nc(cfg, meta):
    NL, G, S, SP, CH = cfg.NL, cfg.G, cfg.S, cfg.SP, cfg.CH
    SC, NSC, TE, BT, EB = cfg.SC, cfg.NSC, cfg.TE, cfg.BT, cfg.EB
    HN, L, C, N = cfg.HN, cfg.L, cfg.C, cfg.N
    NB, NBg, TT = meta["NB"], meta["NBg"], meta["TT"]
    tile_g, tile_sc = meta["tile_g"], meta["tile_sc"]
    tile_st, tile_sp = meta["tile_st"], meta["tile_sp"]
    RG = [list(range(C))]

    nc = bacc.Bacc(None, target_bir_lowering=False, num_swdge_queues=4)
    din = {}

    def ext(name, shape, dt=F32):
        din[name] = nc.dram_tensor(name, shape, dt, kind="ExternalInput")

    ext("xT", [cfg.IN_N, NL], BF16)
    ext("gidx", [128, TT * TE // 16], I16)
    ext("odh", [NB, 128, EB], BF16)
    ext("odth", [NB, 128, BT * 128], FP8)
    ext("fnw", [cfg.IN_N, HN]); ext("fnb", [HN, 1])
    ext("gam", [HN, 1]); ext("bet", [HN, 1])
    ext("fcw", [HN, L, HN], BF16); ext("fcb", [HN, L])
    ext("wsrc", [HN, L, HN], BF16); ext("wdst", [HN, L, HN], BF16)
    ext("adb", [HN, L])
    ext("urow", [L, HN], BF16)
    ext("urow_b", [L, NSC, HN], BF16)
    ext("identb", [128, 128], BF16)

    out = nc.dram_tensor("out", [NL, HN], F32, kind="ExternalOutput")
    lt = [nc.dram_tensor(f"lt{s}", [SP, 128], BF16) for s in range(G)]
    fsh = [nc.dram_tensor(f"fsh{s}", [C, SP, 128], BF16, addr_space="Shared")
           for s in range(G)]
    bn_loc = nc.dram_tensor("bn_loc", [HN, 2], F32)
    bn_sh = nc.dram_tensor("bn_sh", [HN, 2], F32, addr_space="Shared")

    with tile.TileContext(nc) as tc:
        with tc.tile_pool(name="persist", bufs=1) as pp:
            nc.gpsimd.load_library(libcfg.mlp)
            w = {}
            for nm, shp, dt in (
                ("fnw", [cfg.IN_N, HN], F32), ("fnb", [HN, 1], F32),
                ("gam", [HN, 1], F32), ("bet", [HN, 1], F32),
                ("fcw", [HN, L, HN], BF16), ("fcb", [HN, L], F32),
                ("wsrc", [HN, L, HN], BF16), ("wdst", [HN, L, HN], BF16),
                ("adb", [HN, L], F32), ("urow", [L, HN], BF16),
                ("identb", [128, 128], BF16),
            ):
                w[nm] = pp.tile(shp, dt, tag=nm, name=nm)
                nc.sync.dma_start(out=w[nm][:], in_=din[nm][:])
            ident = w["identb"]
            hTb = pp.tile([HN, NL], BF16, tag="hTb", name="hTb")
            ad_nm = pp.tile([128, NSC, HN], BF16, tag="ad_nm", name="ad_nm")
            accum = pp.tile([128, NSC, 128], F32, tag="accum", name="accum")

            # ---------------- input proj + BN (f32) ----------------
            with tc.tile_pool(name="bn", bufs=2) as bp, \
                 tc.tile_pool(name="bnsq", bufs=1) as bq, \
                 tc.tile_pool(name="bnp", bufs=2, space="PSUM") as bpp:
                xt = bq.tile([cfg.IN_N, NL], BF16, tag="xt")
                nc.sync.dma_start(out=xt[:], in_=din["xT"][:])
                fnwb = bp.tile([cfg.IN_N, HN], BF16, tag="fnwb")
                nc.vector.tensor_copy(fnwb[:], w["fnw"][:])
                for ci in range(-(-NL // 512)):
                    c0 = ci * 512
                    n = min(512, NL - c0)
                    ps = bpp.tile([HN, 512], F32, tag="ps")
                    nc.tensor.matmul(out=ps[:, :n], lhsT=fnwb[:],
                                     rhs=xt[:, c0:c0 + n], start=True, stop=True)
                    nc.vector.tensor_scalar_add(hTb[:, c0:c0 + n], ps[:, :n],
                                                w["fnb"][:])
                st = bp.tile([HN, 2], F32, tag="st")
                stp = bp.tile([HN, cfg.G], F32, tag="stp")
                sqc = bq.tile([HN, S], F32, tag="sqc")
                nc.vector.reduce_sum(st[:, 0:1], hTb[:], axis=mybir.AxisListType.X)
                for j in range(cfg.G):
                    nc.vector.scalar_tensor_tensor(
                        out=sqc[:], in0=hTb[:, j * S:(j + 1) * S], scalar=1.0,
                        in1=hTb[:, j * S:(j + 1) * S], op0=AX.mult, op1=AX.mult,
                        accum_out=stp[:, j:j + 1])
                nc.vector.reduce_sum(st[:, 1:2], stp[:], axis=mybir.AxisListType.X)
                nc.sync.dma_start(out=bn_loc[:], in_=st[:])
                nc.gpsimd.collective_compute("AllReduce", AX.add,
                                             replica_groups=RG,
                                             ins=[bn_loc[:]], outs=[bn_sh[:]])
                sg = bp.tile([HN, 2], F32, tag="sg")
                nc.sync.dma_start(out=sg[:], in_=bn_sh[:])
                mean = bp.tile([HN, 1], F32, tag="mean")
                var = bp.tile([HN, 1], F32, tag="var")
                nc.vector.tensor_scalar_mul(mean[:], sg[:, 0:1], 1.0 / N)
                nc.vector.tensor_scalar_mul(var[:], sg[:, 1:2], 1.0 / N)
                msq = bp.tile([HN, 1], F32, tag="msq")
                nc.vector.tensor_mul(msq[:], mean[:], mean[:])
                nc.vector.tensor_sub(var[:], var[:], msq[:])
                nc.vector.tensor_scalar_add(var[:], var[:], cfg.EPS)
                rs = bp.tile([HN, 1], F32, tag="rs")
                nc.scalar.activation(out=rs[:], in_=var[:], func=AF.Sqrt)
                nc.vector.reciprocal(rs[:], rs[:])
                scale = bp.tile([HN, 1], F32, tag="scale")
                nc.vector.tensor_mul(scale[:], rs[:], w["gam"][:])
                nbias = bp.tile([HN, 1], F32, tag="nbias")
                nc.vector.tensor_mul(nbias[:], mean[:], scale[:])
                nc.vector.scalar_tensor_tensor(out=nbias[:], in0=nbias[:],
                                               scalar=-1.0, in1=w["bet"][:],
                                               op0=AX.mult, op1=AX.add)
                nc.vector.tensor_scalar(out=hTb[:], in0=hTb[:], scalar1=scale[:],
                                        scalar2=nbias[:], op0=AX.mult, op1=AX.add)

            # ---------------- layers ----------------
            for l in range(L):
                # --- table build ---
                with tc.tile_pool(name=f"tb{l}", bufs=2) as tp, \
                     tc.tile_pool(name=f"tq{l}", bufs=1) as tq, \
                     tc.tile_pool(name=f"tp{l}", bufs=2, space="PSUM") as tpp:
                    adT = tq.tile([HN, NL], BF16, tag="adT")
                    nc.vector.memset(ad_nm[:], 0.0)
                    for s in range(G):
                        hls = tp.tile([HN, S], BF16, tag="hls")
                        asl = tp.tile([HN, S], BF16, tag="asl")
                        for ci in range(-(-S // 512)):
                            c0 = ci * 512
                            n = min(512, S - c0)
                            a0 = s * S + c0
                            ph = tpp.tile([HN, 512], F32, tag="ph")
                            nc.tensor.matmul(out=ph[:, :n], lhsT=w["fcw"][:, l, :],
                                             rhs=hTb[:, a0:a0 + n],
                                             start=True, stop=True)
                            nc.vector.tensor_scalar_add(hls[:, c0:c0 + n],
                                                        ph[:, :n],
                                                        w["fcb"][:, l:l + 1])
                            psa = tpp.tile([HN, 512], F32, tag="ph")
                            nc.tensor.matmul(out=psa[:, :n], lhsT=w["wsrc"][:, l, :],
                                             rhs=hls[:, c0:c0 + n],
                                             start=True, stop=True)
                            nc.scalar.copy(out=asl[:, c0:c0 + n], in_=psa[:, :n])
                            psd = tpp.tile([HN, 512], F32, tag="ph")
                            nc.tensor.matmul(out=psd[:, :n], lhsT=w["wdst"][:, l, :],
                                             rhs=hls[:, c0:c0 + n],
                                             start=True, stop=True)
                            nc.vector.tensor_scalar_add(adT[:, a0:a0 + n],
                                                        psd[:, :n],
                                                        w["adb"][:, l:l + 1])
                        # node-major rows for slice s
                        stg = tp.tile([128, cfg.RP, 128], BF16, tag="stg")
                        nc.vector.memset(stg[:], 0.0)
                        for r in range(cfg.RP):
                            c0 = r * 128
                            n = min(128, S - c0)
                            pt1 = tpp.tile([128, HN], BF16, tag="pt")
                            nc.tensor.transpose(out=pt1[:n, :],
                                                in_=asl[:, c0:c0 + n],
                                                identity=ident[:HN, :HN])
                            nc.scalar.copy(out=stg[:n, r, 0:HN], in_=pt1[:n, :])
                            pt2 = tpp.tile([128, HN], BF16, tag="pt")
                            nc.tensor.transpose(out=pt2[:n, :],
                                                in_=hls[:, c0:c0 + n],
                                                identity=ident[:HN, :HN])
                            nc.scalar.copy(out=stg[:n, r, HN:128], in_=pt2[:n, :])
                        nc.sync.dma_start(
                            out=lt[s][:, :].rearrange("(r p) k -> p r k", p=128),
                            in_=stg[:, :, :])
                        nc.gpsimd.collective_compute("AllGather", AX.bypass,
                                                     replica_groups=RG,
                                                     ins=[lt[s][:]],
                                                     outs=[fsh[s][:]])
                    # a_d node-major subtables
                    for sc in range(NSC):
                        c0 = sc * SC
                        n = min(SC, NL - c0)
                        pta = tpp.tile([128, HN], BF16, tag="pt")
                        nc.tensor.transpose(out=pta[:n, :],
                                            in_=adT[:, c0:c0 + n],
                                            identity=ident[:HN, :HN])
                        nc.scalar.copy(out=ad_nm[:n, sc, :], in_=pta[:n, :])
                    nc.sync.dma_start(out=ad_nm[127:128, :, :],
                                      in_=din["urow_b"][l:l + 1, :, :])
                    nc.vector.memset(accum[:], 0.0)

                # --- edge blocks ---
                with tc.tile_pool(name=f"ep{l}", bufs=3) as ep, \
                     tc.tile_pool(name=f"px{l}", bufs=2, space="PSUM") as pxp, \
                     tc.tile_pool(name=f"pa{l}", bufs=4, space="PSUM") as pap:
                    psagg = None
                    for b in range(NB):
                        g = tile_g[b * BT]
                        t_base = b * BT
                        gix = ep.tile([128, EB // 16], I16, tag="gix")
                        nc.scalar.dma_start(
                            out=gix[:],
                            in_=din["gidx"][:, b * (EB // 16):(b + 1) * (EB // 16)])
                        od = ep.tile([128, EB], BF16, tag="od")
                        nc.scalar.dma_start(out=od[:], in_=din["odh"][b, :, :])
                        odt = ep.tile([128, BT, 128], FP8, tag="odt")
                        nc.sync.dma_start(
                            out=odt[:],
                            in_=din["odth"][b, :, :].rearrange(
                                "p (t j) -> p t j", j=128))
                        srcr = ep.tile([128, BT, 128], BF16, tag="srcr")
                        NQ = 4 if BT % 4 == 0 else 1
                        for hh in range(NQ):
                            nc.gpsimd.dma_gather(
                                out_ap=srcr[:, hh * (BT // NQ):(hh + 1) * (BT // NQ), :],
                                in_ap=fsh[g][:].rearrange("c s k -> (c s) k"),
                                idxs_ap=gix[:, hh * (EB // (16 * NQ)):(hh + 1) * (EB // (16 * NQ))],
                                num_idxs=EB // NQ, num_idxs_reg=EB // NQ,
                                elem_size=128,
                                single_packet=False,
                                queue_num=(NQ * b + hh) % 4)
                        # expand + alpha, pipelined at SUB granularity
                        SUB = cfg.SUB
                        contrib = ep.tile([128, BT, 128], BF16, tag="contrib")
                        for ww in range(BT // SUB):
                            ws = slice(ww * SUB, (ww + 1) * SUB)
                            psx = pxp.tile([128, SUB, HN], F32, tag="psx")
                            for t2 in range(SUB):
                                t = t_base + ww * SUB + t2
                                nc.tensor.matmul(
                                    out=psx[:, t2, :],
                                    lhsT=od[:, (ww * SUB + t2) * TE:(ww * SUB + t2 + 1) * TE],
                                    rhs=ad_nm[:, tile_sc[t], :],
                                    start=True, stop=True)
                            nc.vector.scalar_tensor_tensor(
                                out=contrib[:, ws, HN:128],
                                in0=psx[:], scalar=1.0,
                                in1=srcr[:, ws, 0:HN],
                                op0=AX.mult, op1=AX.add)
                            nc.vector.scalar_tensor_tensor(
                                out=contrib[:, ws, HN:128],
                                in0=contrib[:, ws, HN:128], scalar=cfg.NEG,
                                in1=contrib[:, ws, HN:128],
                                op0=AX.mult, op1=AX.max)
                            nc.scalar.activation(out=contrib[:, ws, HN:128],
                                                 in_=contrib[:, ws, HN:128],
                                                 func=AF.Exp)
                            nc.vector.tensor_mul(contrib[:, ws, 0:HN],
                                                 contrib[:, ws, HN:128],
                                                 srcr[:, ws, HN:128])
                        # aggregation
                        for t2 in range(BT):
                            t = t_base + t2
                            if tile_st[t]:
                                psagg = pap.tile([128, 128], F32, tag="psagg")
                            nc.tensor.matmul(out=psagg[:],
                                             lhsT=odt[:, t2, :],
                                             rhs=contrib[:, t2, :],
                                             start=bool(tile_st[t]),
                                             stop=bool(tile_sp[t]))
                            if tile_sp[t]:
                                sc = tile_sc[t]
                                nc.vector.tensor_add(accum[:, sc, :],
                                                     accum[:, sc, :], psagg[:])

                # --- finalize ---
                with tc.tile_pool(name=f"fi{l}", bufs=2) as fp, \
                     tc.tile_pool(name=f"fp{l}", bufs=2, space="PSUM") as fpp:
                    den = fp.tile([128, NSC, HN], F32, tag="den")
                    nc.vector.tensor_scalar_add(den[:], accum[:, :, HN:128], 1e-16)
                    nc.vector.reciprocal(den[:], den[:])
                    if l < L - 1:
                        hnm = fp.tile([128, NSC, HN], BF16, tag="hnm")
                        nc.vector.tensor_mul(hnm[:], accum[:, :, 0:HN], den[:])
                        for sc in range(NSC):
                            c0 = sc * SC
                            n = min(SC, NL - c0)
                            pth = fpp.tile([HN, 128], BF16, tag="pth")
                            nc.tensor.transpose(out=pth[:], in_=hnm[:, sc, :],
                                                identity=ident[:, :])
                            nc.scalar.copy(out=hTb[:, c0:c0 + n], in_=pth[:, :n])
                    else:
                        hno = fp.tile([128, NSC, HN], F32, tag="hno")
                        nc.vector.tensor_mul(hno[:], accum[:, :, 0:HN], den[:])
                        nfull = (NSC - 1) * SC       # 12446
                        nc.sync.dma_start(
                            out=out[0:nfull, :].rearrange("(s d) k -> d s k", d=SC),
                            in_=hno[0:SC, 0:NSC - 1, :])
                        nc.sync.dma_start(
                            out=out[nfull:NL, :],
                            in_=hno[0:NL - nfull, NSC - 1, :])
    return nc


# --------------------------------------------------------------- entry ------
def kernel(**inputs):
    cfg = Cfg()
    in_maps, meta = host_prepare(cfg, inputs)
    nc = build_nc(cfg, meta)
    nc.compile()
    split_sync_waits(nc)
    res = run_bass_kernel_spmd(nc, in_maps, core_ids=list(range(cfg.C)),
                               trace=bool(int(os.environ.get("GNN_TRACE", "0"))))
    if res.exec_time_ns is not None:
        print(f"HW exec time: {res.exec_time_ns} ns")
    h = np.concatenate([np.asarray(res.results[c]["out"])
                        for c in range(cfg.C)], axis=0)
    return h.reshape(cfg.B, -1, cfg.HN)
